# revision 5
# baseline (speedup 1.0000x reference)
"""Trainium2 Bass kernel for nn_Comm_OUT (Linear+BN+PReLU -> 20-step GRU ->
BN+PReLU -> 1x1 conv -> BN+PReLU -> linear head), data-parallel over 8 cores.

Layout strategy: everything on-chip is kept "transposed" (channels on SBUF
partitions, batch on the free dim) so the GRU recurrence never needs an
on-chip transpose:
    gh.T[3H, B] = W_hh @ h.T   (W_hh.T tiles are the stationary operand)
The per-step `gi + gh` adds are folded into the PE via identity-matmul PSUM
preloads, biases are folded into per-partition activation bias operands, and
BN+PReLU collapses into single Prelu activations.
"""

import numpy as np
import ml_dtypes

import concourse.bacc as bacc
import concourse.mybir as mybir
import concourse.tile as tile
from concourse import bass_utils

AF = mybir.ActivationFunctionType
OP = mybir.AluOpType
F32 = mybir.dt.float32
F32R = mybir.dt.float32r
BF16 = mybir.dt.bfloat16

E, S, F, H, C, L = 64, 128, 640, 256, 32, 20
EPS = 1e-5
NCORES = 8
B = E * S              # 8192
BC = B // NCORES       # 1024 batch rows per core
NCH = 2                # chunks per core
CB = BC // NCH         # 512 batch rows per chunk (PSUM-bank friendly)

# --- precision config -------------------------------------------------------
# GATE_BF16: store gates/hidden state (and conv/W_mu operands) in bf16 for 2x
# DVE tensor_tensor throughput. Matmuls otherwise run fp32 bitcast to fp32r
# (full PE speed at N>=256).
GATE_BF16 = True

# const-vector column indices (packed [128, NV] tensor, one column per
# per-partition operand vector)
CV_S1, CV_T1, CV_GIB, CV_BHN, CV_S2, CV_T2, CV_S3, CV_T3 = 0, 2, 4, 10, 12, 14, 16, 18
CV_BMU, CV_A1, CV_A2, CV_A3 = 20, 21, 22, 23
NV = 24

_CACHE: dict = {}


def _gdt():
    return BF16 if GATE_BF16 else F32


def _np_gdt():
    return ml_dtypes.bfloat16 if GATE_BF16 else np.float32


def _mm(x):
    """bitcast fp32 APs to fp32r for full-rate PE; bf16 passes through."""
    if x.dtype == F32:
        return x.bitcast(F32R)
    return x


def build_program():
    gdt = _gdt()
    nc = bacc.Bacc("TRN2", target_bir_lowering=False, debug=False)

    xT_h = nc.dram_tensor("xT", [F, BC], F32R, kind="ExternalInput")
    wlin_h = nc.dram_tensor("wlin", [F, H], F32R, kind="ExternalInput")
    wih_h = nc.dram_tensor("wih", [H, 3 * H], F32R, kind="ExternalInput")
    whh_h = nc.dram_tensor("whh", [H, 3 * H], gdt, kind="ExternalInput")
    wc_h = nc.dram_tensor("wc", [H, H], gdt, kind="ExternalInput")
    wmu_h = nc.dram_tensor("wmu", [H, C], gdt, kind="ExternalInput")
    cv_h = nc.dram_tensor("cv", [128, NV], F32, kind="ExternalInput")
    idt_h = nc.dram_tensor("idt", [128, 128], F32R, kind="ExternalInput")
    out_h = nc.dram_tensor("out", [BC, L * C], F32, kind="ExternalOutput")

    with tile.TileContext(nc) as tc:
        with (
            tc.tile_pool(name="consts", bufs=1) as cpool,
            tc.tile_pool(name="gi", bufs=1) as gip,
            tc.tile_pool(name="hp", bufs=2) as hp,
            tc.tile_pool(name="gates", bufs=3) as gp,
            tc.tile_pool(name="s24", bufs=2) as sp,
            tc.tile_pool(name="ps2", bufs=3, space="PSUM") as ps2,
            tc.tile_pool(name="ps1", bufs=2, space="PSUM") as ps1,
        ):
            cvt = cpool.tile([128, NV], F32, tag="cv")
            nc.sync.dma_start(cvt[:], cv_h[:])
            idt = cpool.tile([128, 128], F32R, tag="idt")
            nc.sync.dma_start(idt[:], idt_h[:])
            wih_t = cpool.tile([128, 2 * 3 * H], F32R, tag="wih")
            whh_t = cpool.tile([128, 2 * 3 * H], gdt, tag="whh")
            for k in range(2):
                nc.sync.dma_start(
                    wih_t[:, k * 3 * H : (k + 1) * 3 * H],
                    wih_h[k * 128 : (k + 1) * 128, :],
                )
                nc.sync.dma_start(
                    whh_t[:, k * 3 * H : (k + 1) * 3 * H],
                    whh_h[k * 128 : (k + 1) * 128, :],
                )
            wc_t = cpool.tile([128, 2 * H], gdt, tag="wc")
            wmu_t = cpool.tile([128, 2 * C], gdt, tag="wmu")
            for k in range(2):
                nc.sync.dma_start(
                    wc_t[:, k * H : (k + 1) * H], wc_h[k * 128 : (k + 1) * 128, :]
                )
                nc.sync.dma_start(
                    wmu_t[:, k * C : (k + 1) * C], wmu_h[k * 128 : (k + 1) * 128, :]
                )

            def pp(col):  # per-partition operand column
                return cvt[:, col : col + 1]

            # ---- stage 1: x1 = prelu(bn(x @ W_lin.T)), gi = x1 @ W_ih.T ----
            gi_tiles = []
            with tc.tile_pool(name="stage1", bufs=1) as xp:
                xt = xp.tile([128, 5 * BC], F32R, tag="xT")
                for k in range(5):
                    nc.sync.dma_start(
                        xt[:, k * BC : (k + 1) * BC], xT_h[k * 128 : (k + 1) * 128, :]
                    )
                wlin_t = xp.tile([128, 5 * H], F32R, tag="wlin")
                for k in range(5):
                    nc.sync.dma_start(
                        wlin_t[:, k * H : (k + 1) * H], wlin_h[k * 128 : (k + 1) * 128, :]
                    )
                for c in range(NCH):
                    ps_x1 = ps2.tile([128, 2 * CB], F32, tag="g")
                    for k in range(5):
                        for m in range(2):
                            nc.tensor.matmul(
                                ps_x1[:, m * CB : (m + 1) * CB],
                                wlin_t[:, k * H + m * 128 : k * H + (m + 1) * 128],
                                xt[:, k * BC + c * CB : k * BC + (c + 1) * CB],
                                start=(k == 0),
                                stop=(k == 4),
                            )
                    x1p = xp.tile([128, 2 * CB], F32R, tag=f"x1p{c}")
                    for m in range(2):
                        nc.scalar.activation(
                            x1p[:, m * CB : (m + 1) * CB],
                            ps_x1[:, m * CB : (m + 1) * CB],
                            AF.Prelu,
                            bias=pp(CV_T1 + m),
                            scale=pp(CV_S1 + m),
                            alpha=pp(CV_A1),
                        )
                    gi = gip.tile([128, 6 * CB], F32R, tag=f"gi{c}")
                    gi_tiles.append(gi)
                    for m in range(6):
                        ps_gi = ps1.tile([128, CB], F32, tag="p")
                        for k in range(2):
                            nc.tensor.matmul(
                                ps_gi[:],
                                wih_t[:, k * 3 * H + m * 128 : k * 3 * H + (m + 1) * 128],
                                x1p[:, k * CB : (k + 1) * CB],
                                start=(k == 0),
                                stop=(k == 1),
                            )
                        # gi' = gi + (b_ih [+ b_hh for r,z]) folded via bias
                        nc.scalar.activation(
                            gi[:, m * CB : (m + 1) * CB],
                            ps_gi[:],
                            AF.Identity,
                            bias=pp(CV_GIB + m),
                        )

            # ---- GRU + stages 2..4, per chunk ----
            for c in range(NCH):
                gi = gi_tiles[c]
                h = hp.tile([128, 2 * CB], gdt, tag=f"h{c}")
                nc.vector.memset(h[:], 0.0)
                ps_pack = None
                for t in range(L):
                    ps_r = ps2.tile([128, 2 * CB], F32, tag="g")
                    ps_z = ps2.tile([128, 2 * CB], F32, tag="g")
                    ps_n = ps2.tile([128, 2 * CB], F32, tag="g")
                    # identity preloads inject gi' for the r,z gates
                    for g, ps in ((0, ps_r), (1, ps_z)):
                        for m in range(2):
                            nc.tensor.matmul(
                                ps[:, m * CB : (m + 1) * CB],
                                idt[:],
                                gi[:, (2 * g + m) * CB : (2 * g + m + 1) * CB],
                                start=True,
                                stop=False,
                            )
                    for k in range(2):
                        for g, ps in ((0, ps_r), (1, ps_z), (2, ps_n)):
                            for m in range(2):
                                row = 2 * g + m
                                nc.tensor.matmul(
                                    ps[:, m * CB : (m + 1) * CB],
                                    whh_t[:, k * 3 * H + row * 128 : k * 3 * H + (row + 1) * 128],
                                    h[:, k * CB : (k + 1) * CB],
                                    start=(g == 2 and k == 0),
                                    stop=(k == 1),
                                )
                    r_sb = gp.tile([128, 2 * CB], gdt, tag="r")
                    nc.scalar.activation(r_sb[:], ps_r[:], AF.Sigmoid)
                    z_sb = gp.tile([128, 2 * CB], gdt, tag="z")
                    nc.scalar.activation(z_sb[:], ps_z[:], AF.Sigmoid)
                    # t = (gh_n + b_hh_n) * r
                    t_sb = gp.tile([128, 2 * CB], gdt, tag="t")
                    for m in range(2):
                        nc.vector.scalar_tensor_tensor(
                            t_sb[:, m * CB : (m + 1) * CB],
                            ps_n[:, m * CB : (m + 1) * CB],
                            pp(CV_BHN + m),
                            r_sb[:, m * CB : (m + 1) * CB],
                            op0=OP.add,
                            op1=OP.mult,
                        )
                    a_sb = gp.tile([128, 2 * CB], gdt, tag="a")
                    nc.vector.tensor_tensor(a_sb[:], t_sb[:], gi[:, 4 * CB : 6 * CB].bitcast(F32), OP.add)
                    n_sb = gp.tile([128, 2 * CB], gdt, tag="n")
                    nc.scalar.activation(n_sb[:], a_sb[:], AF.Tanh)
                    # h' = n + z*(h - n)
                    u_sb = gp.tile([128, 2 * CB], gdt, tag="u")
                    nc.vector.tensor_tensor(u_sb[:], h[:], n_sb[:], OP.subtract)
                    v_sb = gp.tile([128, 2 * CB], gdt, tag="v")
                    nc.vector.tensor_tensor(v_sb[:], z_sb[:], u_sb[:], OP.mult)
                    h = hp.tile([128, 2 * CB], gdt, tag=f"h{c}")
                    nc.vector.tensor_tensor(h[:], n_sb[:], v_sb[:], OP.add)

                    # ---- stage 2: w2 = prelu(bn2(h_t)) on DVE ----
                    w2a = gp.tile([128, 2 * CB], gdt, tag="w2a")
                    for m in range(2):
                        nc.vector.tensor_scalar(
                            w2a[:, m * CB : (m + 1) * CB],
                            h[:, m * CB : (m + 1) * CB],
                            pp(CV_S2 + m),
                            pp(CV_T2 + m),
                            op0=OP.mult,
                            op1=OP.add,
                        )
                    w2 = gp.tile([128, 2 * CB], gdt, tag="w2")
                    # prelu(x) = max(a*x, x) for 0 <= a <= 1
                    nc.vector.scalar_tensor_tensor(
                        w2[:], w2a[:], pp(CV_A2), w2a[:], op0=OP.mult, op1=OP.max
                    )
                    # ---- stage 3: conv (k=1) + bn3 + prelu ----
                    ps_cv = ps2.tile([128, 2 * CB], F32, tag="g")
                    for k in range(2):
                        for m in range(2):
                            nc.tensor.matmul(
                                ps_cv[:, m * CB : (m + 1) * CB],
                                wc_t[:, k * H + m * 128 : k * H + (m + 1) * 128],
                                w2[:, k * CB : (k + 1) * CB],
                                start=(k == 0),
                                stop=(k == 1),
                            )
                    y2 = gp.tile([128, 2 * CB], gdt, tag="y2")
                    for m in range(2):
                        nc.scalar.activation(
                            y2[:, m * CB : (m + 1) * CB],
                            ps_cv[:, m * CB : (m + 1) * CB],
                            AF.Prelu,
                            bias=pp(CV_T3 + m),
                            scale=pp(CV_S3 + m),
                            alpha=pp(CV_A3),
                        )
                    # ---- stage 4: W_mu head, col-tiled 4 steps into one bank ----
                    j = t % 4
                    if j == 0:
                        ps_pack = ps1.tile([128, CB], F32, tag="p")
                    for k in range(2):
                        nc.tensor.matmul(
                            ps_pack[32 * j : 32 * (j + 1), :],
                            wmu_t[:, k * C : (k + 1) * C],
                            y2[:, k * CB : (k + 1) * CB],
                            start=(k == 0),
                            stop=(k == 1),
                            tile_position=(0, 32 * j),
                        )
                    if j == 3:
                        pk = sp.tile([128, CB], F32, tag="pk")
                        nc.scalar.activation(pk[:], ps_pack[:], AF.Identity, bias=pp(CV_BMU))
                        tr = sp.tile([128, CB], F32, tag="tr")
                        nc.vector.transpose(tr[:], pk[:])
                        l4 = t // 4
                        dst = out_h[:].rearrange(
                            "(c bh bl) (l4 li cc) -> c l4 li bl bh cc",
                            c=NCH, bh=CB // 32, bl=32, l4=L // 4, li=4, cc=C,
                        )
                        for li in range(4):
                            nc.sync.dma_start(
                                dst[c, l4, li],
                                tr[32 * li : 32 * (li + 1), :].rearrange(
                                    "p (bh cc) -> p bh cc", bh=CB // 32
                                ),
                            )

    nc.compile()
    return nc


def _prep_inputs(inputs):
    f32 = np.float32
    gnp = _np_gdt()
    x = np.ascontiguousarray(np.asarray(inputs["h_w_action"], f32).reshape(B, F))
    W_lin = np.asarray(inputs["W_lin"], f32)
    b_lin = np.asarray(inputs["b_lin"], f32)
    W_ih = np.asarray(inputs["W_ih"], f32)
    W_hh = np.asarray(inputs["W_hh"], f32)
    b_ih = np.asarray(inputs["b_ih"], f32)
    b_hh = np.asarray(inputs["b_hh"], f32)
    Wc = np.asarray(inputs["Wc"], f32)
    bc = np.asarray(inputs["bc"], f32)
    W_mu = np.asarray(inputs["W_mu"], f32)
    b_mu = np.asarray(inputs["b_mu"], f32)

    def bnfold(g, beta, m, v):
        s = g / np.sqrt(v + EPS)
        return s, beta - m * s

    s1, t1 = bnfold(inputs["g1"], inputs["beta1"], inputs["m1"], inputs["v1"])
    s2, t2 = bnfold(inputs["g2"], inputs["beta2"], inputs["m2"], inputs["v2"])
    s3, t3 = bnfold(inputs["g3"], inputs["beta3"], inputs["m3"], inputs["v3"])
    s1, t1, s2, t2, s3, t3 = (np.asarray(a, f32) for a in (s1, t1, s2, t2, s3, t3))
    t1 = t1 + s1 * b_lin          # fold linear bias into bn1 shift
    t3 = t3 + s3 * bc             # fold conv bias into bn3 shift
    gib = b_ih.copy()
    gib[: 2 * H] += b_hh[: 2 * H]  # fold b_hh into gi for the r,z gates
    bhn = b_hh[2 * H :]

    cv = np.zeros((128, NV), f32)
    for col, vec in ((CV_S1, s1), (CV_T1, t1), (CV_S2, s2), (CV_T2, t2),
                     (CV_S3, s3), (CV_T3, t3)):
        cv[:, col] = vec[:128]
        cv[:, col + 1] = vec[128:]
    for m in range(6):
        cv[:, CV_GIB + m] = gib[m * 128 : (m + 1) * 128]
    cv[:, CV_BHN] = bhn[:128]
    cv[:, CV_BHN + 1] = bhn[128:]
    cv[:, CV_BMU] = np.tile(b_mu, 4)
    cv[:, CV_A1] = f32(np.asarray(inputs["a1"]).reshape(-1)[0])
    cv[:, CV_A2] = f32(np.asarray(inputs["a2"]).reshape(-1)[0])
    cv[:, CV_A3] = f32(np.asarray(inputs["a3"]).reshape(-1)[0])

    shared = {
        "wlin": np.ascontiguousarray(W_lin.T),
        "wih": np.ascontiguousarray(W_ih.T),
        "whh": np.ascontiguousarray(W_hh.T).astype(gnp),
        "wc": np.ascontiguousarray(Wc.T).astype(gnp),
        "wmu": np.ascontiguousarray(W_mu.T).astype(gnp),
        "cv": cv,
        "idt": np.eye(128, dtype=f32),
    }
    in_maps = []
    for i in range(NCORES):
        m = dict(shared)
        m["xT"] = np.ascontiguousarray(x[i * BC : (i + 1) * BC, :].T)
        in_maps.append(m)
    return in_maps


def kernel(**inputs) -> np.ndarray:
    if "nc" not in _CACHE:
        _CACHE["nc"] = build_program()
    nc = _CACHE["nc"]
    in_maps = _prep_inputs(inputs)
    res = bass_utils.run_bass_kernel_spmd(nc, in_maps, core_ids=list(range(NCORES)))
    outs = [np.asarray(r["out"], np.float32) for r in res.results]
    return np.concatenate(outs, axis=0).reshape(E, S, L, C)


# revision 10
# speedup vs baseline: 1.0386x; 1.0386x over previous
"""Trainium2 Bass kernel for nn_Comm_OUT (Linear+BN+PReLU -> 20-step GRU ->
BN+PReLU -> 1x1 conv -> BN+PReLU -> linear head), data-parallel over 8 cores.

Layout strategy: everything on-chip is kept "transposed" (channels on SBUF
partitions, batch on the free dim) so the GRU recurrence never needs an
on-chip transpose:
    gh.T[3H, B] = W_hh @ h.T   (W_hh.T tiles are the stationary operand)
The per-step `gi + gh` adds are folded into the PE via identity-matmul PSUM
preloads, biases are folded into per-partition activation bias operands, and
BN+PReLU collapses into single Prelu activations.
"""

import numpy as np
import ml_dtypes

import concourse.bacc as bacc
import concourse.mybir as mybir
import concourse.tile as tile
from concourse import bass_utils

AF = mybir.ActivationFunctionType
OP = mybir.AluOpType
F32 = mybir.dt.float32
F32R = mybir.dt.float32r
BF16 = mybir.dt.bfloat16

E, S, F, H, C, L = 64, 128, 640, 256, 32, 20
EPS = 1e-5
NCORES = 8
B = E * S              # 8192
BC = B // NCORES       # 1024 batch rows per core
NCH = 2                # chunks per core
CB = BC // NCH         # 512 batch rows per chunk (PSUM-bank friendly)

# --- precision config -------------------------------------------------------
# GATE_BF16: store gates/hidden state (and conv/W_mu operands) in bf16 for 2x
# DVE tensor_tensor throughput. Matmuls otherwise run fp32 bitcast to fp32r
# (full PE speed at N>=256).
GATE_BF16 = True

# const-vector column indices (packed [128, NV] tensor, one column per
# per-partition operand vector)
CV_S1, CV_T1, CV_GIB, CV_BHN, CV_S2, CV_T2, CV_S3, CV_T3 = 0, 2, 4, 10, 12, 14, 16, 18
CV_BMU, CV_A1, CV_A2, CV_A3 = 20, 21, 22, 23
NV = 24

_CACHE: dict = {}


def _gdt():
    return BF16 if GATE_BF16 else F32


def _np_gdt():
    return ml_dtypes.bfloat16 if GATE_BF16 else np.float32


def _mm(x):
    """bitcast fp32 APs to fp32r for full-rate PE; bf16 passes through."""
    if x.dtype == F32:
        return x.bitcast(F32R)
    return x


def build_program():
    gdt = _gdt()
    nc = bacc.Bacc("TRN2", target_bir_lowering=False, debug=False)

    xT_h = nc.dram_tensor("xT", [F, BC], F32R, kind="ExternalInput")
    wlin_h = nc.dram_tensor("wlin", [F, H], F32R, kind="ExternalInput")
    wih_h = nc.dram_tensor("wih", [H, 3 * H], F32R, kind="ExternalInput")
    whh_h = nc.dram_tensor("whh", [H, 3 * H], gdt, kind="ExternalInput")
    wc_h = nc.dram_tensor("wc", [H, H], gdt, kind="ExternalInput")
    wmu_h = nc.dram_tensor("wmu", [H, C], gdt, kind="ExternalInput")
    cv_h = nc.dram_tensor("cv", [128, NV], F32, kind="ExternalInput")
    idt_h = nc.dram_tensor("idt", [128, 128], F32R, kind="ExternalInput")
    out_h = nc.dram_tensor("out", [BC, L * C], F32, kind="ExternalOutput")

    with tile.TileContext(nc) as tc:
        with (
            tc.tile_pool(name="consts", bufs=1) as cpool,
            tc.tile_pool(name="gi", bufs=1) as gip,
            tc.tile_pool(name="hp", bufs=2) as hp,
            tc.tile_pool(name="gates", bufs=3) as gp,
            tc.tile_pool(name="s24", bufs=2) as sp,
            tc.tile_pool(name="ps2", bufs=3, space="PSUM") as ps2,
            tc.tile_pool(name="ps1", bufs=2, space="PSUM") as ps1,
        ):
            cvt = cpool.tile([128, NV], F32, tag="cv")
            nc.sync.dma_start(cvt[:], cv_h[:])
            idt = cpool.tile([128, 128], F32R, tag="idt")
            nc.sync.dma_start(idt[:], idt_h[:])
            wih_t = cpool.tile([128, 2 * 3 * H], F32R, tag="wih")
            whh_t = cpool.tile([128, 2 * 3 * H], gdt, tag="whh")
            for k in range(2):
                nc.sync.dma_start(
                    wih_t[:, k * 3 * H : (k + 1) * 3 * H],
                    wih_h[k * 128 : (k + 1) * 128, :],
                )
                nc.sync.dma_start(
                    whh_t[:, k * 3 * H : (k + 1) * 3 * H],
                    whh_h[k * 128 : (k + 1) * 128, :],
                )
            wc_t = cpool.tile([128, 2 * H], gdt, tag="wc")
            wmu_t = cpool.tile([128, 2 * C], gdt, tag="wmu")
            for k in range(2):
                nc.sync.dma_start(
                    wc_t[:, k * H : (k + 1) * H], wc_h[k * 128 : (k + 1) * 128, :]
                )
                nc.sync.dma_start(
                    wmu_t[:, k * C : (k + 1) * C], wmu_h[k * 128 : (k + 1) * 128, :]
                )

            def pp(col):  # per-partition operand column
                return cvt[:, col : col + 1]

            # ---- stage 1: x1 = prelu(bn(x @ W_lin.T)), gi = x1 @ W_ih.T ----
            gi_tiles = []
            with tc.tile_pool(name="stage1", bufs=1) as xp:
                xt = xp.tile([128, 5 * BC], F32R, tag="xT")
                for k in range(5):
                    nc.sync.dma_start(
                        xt[:, k * BC : (k + 1) * BC], xT_h[k * 128 : (k + 1) * 128, :]
                    )
                wlin_t = xp.tile([128, 5 * H], F32R, tag="wlin")
                for k in range(5):
                    nc.sync.dma_start(
                        wlin_t[:, k * H : (k + 1) * H], wlin_h[k * 128 : (k + 1) * 128, :]
                    )
                for c in range(NCH):
                    ps_x1 = ps2.tile([128, 2 * CB], F32, tag="g")
                    for k in range(5):
                        for m in range(2):
                            nc.tensor.matmul(
                                ps_x1[:, m * CB : (m + 1) * CB],
                                wlin_t[:, k * H + m * 128 : k * H + (m + 1) * 128],
                                xt[:, k * BC + c * CB : k * BC + (c + 1) * CB],
                                start=(k == 0),
                                stop=(k == 4),
                            )
                    x1p = xp.tile([128, 2 * CB], F32R, tag=f"x1p{c}")
                    for m in range(2):
                        nc.scalar.activation(
                            x1p[:, m * CB : (m + 1) * CB],
                            ps_x1[:, m * CB : (m + 1) * CB],
                            AF.Prelu,
                            bias=pp(CV_T1 + m),
                            scale=pp(CV_S1 + m),
                            alpha=pp(CV_A1),
                        )
                    gi = gip.tile([128, 6 * CB], F32R, tag=f"gi{c}")
                    gi_tiles.append(gi)
                    for m in range(6):
                        ps_gi = ps1.tile([128, CB], F32, tag="p")
                        for k in range(2):
                            nc.tensor.matmul(
                                ps_gi[:],
                                wih_t[:, k * 3 * H + m * 128 : k * 3 * H + (m + 1) * 128],
                                x1p[:, k * CB : (k + 1) * CB],
                                start=(k == 0),
                                stop=(k == 1),
                            )
                        # gi' = gi + (b_ih [+ b_hh for r,z]) folded via bias
                        nc.scalar.activation(
                            gi[:, m * CB : (m + 1) * CB],
                            ps_gi[:],
                            AF.Identity,
                            bias=pp(CV_GIB + m),
                        )

            # ---- GRU + stages 2..4; chunks interleaved per step so their
            # independent dependency chains overlap across engines ----
            gin_bf = []
            hs = []
            packs = [None] * NCH
            for c in range(NCH):
                g_bf = gip.tile([128, 2 * CB], gdt, tag=f"ginb{c}", name=f"ginb{c}")
                nc.scalar.activation(g_bf[:], gi_tiles[c][:, 4 * CB : 6 * CB], AF.Copy)
                gin_bf.append(g_bf)
                h = hp.tile([128, 2 * CB], gdt, tag=f"h{c}")
                nc.vector.memset(h[:], 0.0)
                hs.append(h)
            for t in range(L):
                for c in range(NCH):
                    gi = gi_tiles[c]
                    h = hs[c]
                    ps_r = ps2.tile([128, 2 * CB], F32, tag="g")
                    ps_z = ps2.tile([128, 2 * CB], F32, tag="g")
                    ps_n = ps2.tile([128, 2 * CB], F32, tag="g")
                    # identity preloads inject gi' for the r,z gates
                    for g, ps in ((0, ps_r), (1, ps_z)):
                        for m in range(2):
                            nc.tensor.matmul(
                                ps[:, m * CB : (m + 1) * CB],
                                idt[:],
                                gi[:, (2 * g + m) * CB : (2 * g + m + 1) * CB],
                                start=True,
                                stop=False,
                            )
                    for k in range(2):
                        for g, ps in ((0, ps_r), (1, ps_z), (2, ps_n)):
                            for m in range(2):
                                row = 2 * g + m
                                nc.tensor.matmul(
                                    ps[:, m * CB : (m + 1) * CB],
                                    whh_t[:, k * 3 * H + row * 128 : k * 3 * H + (row + 1) * 128],
                                    h[:, k * CB : (k + 1) * CB],
                                    start=(g == 2 and k == 0),
                                    stop=(k == 1),
                                )
                    r_sb = gp.tile([128, 2 * CB], gdt, tag="r")
                    nc.scalar.activation(r_sb[:], ps_r[:], AF.Sigmoid)
                    z_sb = gp.tile([128, 2 * CB], gdt, tag="z")
                    nc.scalar.activation(z_sb[:], ps_z[:], AF.Sigmoid)
                    # t = (gh_n + b_hh_n) * r
                    t_sb = gp.tile([128, 2 * CB], gdt, tag="t")
                    for m in range(2):
                        nc.vector.scalar_tensor_tensor(
                            t_sb[:, m * CB : (m + 1) * CB],
                            ps_n[:, m * CB : (m + 1) * CB],
                            pp(CV_BHN + m),
                            r_sb[:, m * CB : (m + 1) * CB],
                            op0=OP.add,
                            op1=OP.mult,
                        )
                    a_sb = gp.tile([128, 2 * CB], gdt, tag="a")
                    nc.vector.tensor_tensor(a_sb[:], t_sb[:], gin_bf[c][:], OP.add)
                    n_sb = gp.tile([128, 2 * CB], gdt, tag="n")
                    nc.scalar.activation(n_sb[:], a_sb[:], AF.Tanh)
                    # h' = n + z*(h - n)
                    u_sb = gp.tile([128, 2 * CB], gdt, tag="u")
                    nc.vector.tensor_tensor(u_sb[:], h[:], n_sb[:], OP.subtract)
                    v_sb = gp.tile([128, 2 * CB], gdt, tag="v")
                    nc.vector.tensor_tensor(v_sb[:], z_sb[:], u_sb[:], OP.mult)
                    h = hp.tile([128, 2 * CB], gdt, tag=f"h{c}")
                    nc.vector.tensor_tensor(h[:], n_sb[:], v_sb[:], OP.add)
                    hs[c] = h

                    # ---- stage 2: w2 = prelu(bn2(h_t)) on DVE ----
                    w2a = gp.tile([128, 2 * CB], gdt, tag="w2a")
                    for m in range(2):
                        nc.vector.tensor_scalar(
                            w2a[:, m * CB : (m + 1) * CB],
                            h[:, m * CB : (m + 1) * CB],
                            pp(CV_S2 + m),
                            pp(CV_T2 + m),
                            op0=OP.mult,
                            op1=OP.add,
                        )
                    w2 = gp.tile([128, 2 * CB], gdt, tag="w2")
                    # prelu(x) = max(a*x, x) for 0 <= a <= 1
                    nc.vector.scalar_tensor_tensor(
                        w2[:], w2a[:], pp(CV_A2), w2a[:], op0=OP.mult, op1=OP.max
                    )
                    # ---- stage 3: conv (k=1) + bn3 + prelu ----
                    ps_cv = ps2.tile([128, 2 * CB], F32, tag="g")
                    for k in range(2):
                        for m in range(2):
                            nc.tensor.matmul(
                                ps_cv[:, m * CB : (m + 1) * CB],
                                wc_t[:, k * H + m * 128 : k * H + (m + 1) * 128],
                                w2[:, k * CB : (k + 1) * CB],
                                start=(k == 0),
                                stop=(k == 1),
                            )
                    y2 = gp.tile([128, 2 * CB], gdt, tag="y2")
                    for m in range(2):
                        nc.scalar.activation(
                            y2[:, m * CB : (m + 1) * CB],
                            ps_cv[:, m * CB : (m + 1) * CB],
                            AF.Prelu,
                            bias=pp(CV_T3 + m),
                            scale=pp(CV_S3 + m),
                            alpha=pp(CV_A3),
                        )
                    # ---- stage 4: W_mu head, col-tiled 4 steps into one bank ----
                    j = t % 4
                    if j == 0:
                        packs[c] = ps1.tile([128, CB], F32, tag="p", name=f"pack{c}")
                    ps_pack = packs[c]
                    for k in range(2):
                        nc.tensor.matmul(
                            ps_pack[32 * j : 32 * (j + 1), :],
                            wmu_t[:, k * C : (k + 1) * C],
                            y2[:, k * CB : (k + 1) * CB],
                            start=(k == 0),
                            stop=(k == 1),
                            tile_position=(0, 32 * j),
                        )
                    if j == 3:
                        pk = sp.tile([128, CB], F32, tag="pk")
                        nc.scalar.activation(pk[:], ps_pack[:], AF.Identity, bias=pp(CV_BMU))
                        tr = sp.tile([128, CB], F32, tag="tr")
                        nc.vector.transpose(tr[:], pk[:])
                        l4 = t // 4
                        dst = out_h[:].rearrange(
                            "(c bh bl) (l4 li cc) -> c l4 li bl bh cc",
                            c=NCH, bh=CB // 32, bl=32, l4=L // 4, li=4, cc=C,
                        )
                        for li in range(4):
                            nc.sync.dma_start(
                                dst[c, l4, li],
                                tr[32 * li : 32 * (li + 1), :].rearrange(
                                    "p (bh cc) -> p bh cc", bh=CB // 32
                                ),
                            )

    nc.compile()
    return nc


def _prep_inputs(inputs):
    f32 = np.float32
    gnp = _np_gdt()
    x = np.ascontiguousarray(np.asarray(inputs["h_w_action"], f32).reshape(B, F))
    W_lin = np.asarray(inputs["W_lin"], f32)
    b_lin = np.asarray(inputs["b_lin"], f32)
    W_ih = np.asarray(inputs["W_ih"], f32)
    W_hh = np.asarray(inputs["W_hh"], f32)
    b_ih = np.asarray(inputs["b_ih"], f32)
    b_hh = np.asarray(inputs["b_hh"], f32)
    Wc = np.asarray(inputs["Wc"], f32)
    bc = np.asarray(inputs["bc"], f32)
    W_mu = np.asarray(inputs["W_mu"], f32)
    b_mu = np.asarray(inputs["b_mu"], f32)

    def bnfold(g, beta, m, v):
        s = g / np.sqrt(v + EPS)
        return s, beta - m * s

    s1, t1 = bnfold(inputs["g1"], inputs["beta1"], inputs["m1"], inputs["v1"])
    s2, t2 = bnfold(inputs["g2"], inputs["beta2"], inputs["m2"], inputs["v2"])
    s3, t3 = bnfold(inputs["g3"], inputs["beta3"], inputs["m3"], inputs["v3"])
    s1, t1, s2, t2, s3, t3 = (np.asarray(a, f32) for a in (s1, t1, s2, t2, s3, t3))
    t1 = t1 + s1 * b_lin          # fold linear bias into bn1 shift
    t3 = t3 + s3 * bc             # fold conv bias into bn3 shift
    gib = b_ih.copy()
    gib[: 2 * H] += b_hh[: 2 * H]  # fold b_hh into gi for the r,z gates
    bhn = b_hh[2 * H :]

    cv = np.zeros((128, NV), f32)
    for col, vec in ((CV_S1, s1), (CV_T1, t1), (CV_S2, s2), (CV_T2, t2),
                     (CV_S3, s3), (CV_T3, t3)):
        cv[:, col] = vec[:128]
        cv[:, col + 1] = vec[128:]
    for m in range(6):
        cv[:, CV_GIB + m] = gib[m * 128 : (m + 1) * 128]
    cv[:, CV_BHN] = bhn[:128]
    cv[:, CV_BHN + 1] = bhn[128:]
    cv[:, CV_BMU] = np.tile(b_mu, 4)
    cv[:, CV_A1] = f32(np.asarray(inputs["a1"]).reshape(-1)[0])
    cv[:, CV_A2] = f32(np.asarray(inputs["a2"]).reshape(-1)[0])
    cv[:, CV_A3] = f32(np.asarray(inputs["a3"]).reshape(-1)[0])

    shared = {
        "wlin": np.ascontiguousarray(W_lin.T),
        "wih": np.ascontiguousarray(W_ih.T),
        "whh": np.ascontiguousarray(W_hh.T).astype(gnp),
        "wc": np.ascontiguousarray(Wc.T).astype(gnp),
        "wmu": np.ascontiguousarray(W_mu.T).astype(gnp),
        "cv": cv,
        "idt": np.eye(128, dtype=f32),
    }
    in_maps = []
    for i in range(NCORES):
        m = dict(shared)
        m["xT"] = np.ascontiguousarray(x[i * BC : (i + 1) * BC, :].T)
        in_maps.append(m)
    return in_maps


def kernel(**inputs) -> np.ndarray:
    if "nc" not in _CACHE:
        _CACHE["nc"] = build_program()
    nc = _CACHE["nc"]
    in_maps = _prep_inputs(inputs)
    res = bass_utils.run_bass_kernel_spmd(nc, in_maps, core_ids=list(range(NCORES)))
    outs = [np.asarray(r["out"], np.float32) for r in res.results]
    return np.concatenate(outs, axis=0).reshape(E, S, L, C)


# revision 11
# speedup vs baseline: 1.4002x; 1.3482x over previous
"""Trainium2 Bass kernel for nn_Comm_OUT (Linear+BN+PReLU -> 20-step GRU ->
BN+PReLU -> 1x1 conv -> BN+PReLU -> linear head), data-parallel over 8 cores.

Layout strategy: everything on-chip is kept "transposed" (channels on SBUF
partitions, batch on the free dim) so the GRU recurrence never needs an
on-chip transpose:
    gh.T[3H, B] = W_hh @ h.T   (W_hh.T tiles are the stationary operand)
The per-step `gi + gh` adds are folded into the PE via identity-matmul PSUM
preloads, biases are folded into per-partition activation bias operands, and
BN+PReLU collapses into single Prelu activations.
"""

import numpy as np
import ml_dtypes

import concourse.bacc as bacc
import concourse.mybir as mybir
import concourse.tile as tile
from concourse import bass_utils

AF = mybir.ActivationFunctionType
OP = mybir.AluOpType
F32 = mybir.dt.float32
F32R = mybir.dt.float32r
BF16 = mybir.dt.bfloat16

E, S, F, H, C, L = 64, 128, 640, 256, 32, 20
EPS = 1e-5
NCORES = 8
B = E * S              # 8192
BC = B // NCORES       # 1024 batch rows per core
NCH = 2                # chunks per core
CB = BC // NCH         # 512 batch rows per chunk (PSUM-bank friendly)

# --- precision config -------------------------------------------------------
# GATE_BF16: store gates/hidden state (and conv/W_mu operands) in bf16 for 2x
# DVE tensor_tensor throughput. Matmuls otherwise run fp32 bitcast to fp32r
# (full PE speed at N>=256).
GATE_BF16 = True

# const-vector column indices (packed [128, NV] tensor, one column per
# per-partition operand vector)
CV_S1, CV_T1, CV_GIB, CV_BHN, CV_S2, CV_T2, CV_S3, CV_T3 = 0, 2, 4, 10, 12, 14, 16, 18
CV_BMU, CV_A1, CV_A2, CV_A3 = 20, 21, 22, 23
NV = 24

_CACHE: dict = {}


def _gdt():
    return BF16 if GATE_BF16 else F32


def _np_gdt():
    return ml_dtypes.bfloat16 if GATE_BF16 else np.float32


def _mm(x):
    """bitcast fp32 APs to fp32r for full-rate PE; bf16 passes through."""
    if x.dtype == F32:
        return x.bitcast(F32R)
    return x


def build_program():
    gdt = _gdt()
    nc = bacc.Bacc("TRN2", target_bir_lowering=False, debug=False)

    xT_h = nc.dram_tensor("xT", [F, BC], F32R, kind="ExternalInput")
    wlin_h = nc.dram_tensor("wlin", [F, H], F32R, kind="ExternalInput")
    wih_h = nc.dram_tensor("wih", [H, 3 * H], F32R, kind="ExternalInput")
    whh_h = nc.dram_tensor("whh", [H, 3 * H], gdt, kind="ExternalInput")
    wc_h = nc.dram_tensor("wc", [H, H], gdt, kind="ExternalInput")
    wmu_h = nc.dram_tensor("wmu", [H, C], gdt, kind="ExternalInput")
    cv_h = nc.dram_tensor("cv", [128, NV], F32, kind="ExternalInput")
    idt_h = nc.dram_tensor("idt", [128, 128], F32R, kind="ExternalInput")
    out_h = nc.dram_tensor("out", [BC, L * C], F32, kind="ExternalOutput")

    with tile.TileContext(nc) as tc:
        with (
            tc.tile_pool(name="consts", bufs=1) as cpool,
            tc.tile_pool(name="gi", bufs=1) as gip,
            tc.tile_pool(name="hp", bufs=2) as hp,
            tc.tile_pool(name="gates", bufs=3) as gp,
            tc.tile_pool(name="s24", bufs=2) as sp,
            tc.tile_pool(name="ps2", bufs=3, space="PSUM") as ps2,
            tc.tile_pool(name="ps1", bufs=2, space="PSUM") as ps1,
        ):
            cvt = cpool.tile([128, NV], F32, tag="cv")
            nc.sync.dma_start(cvt[:], cv_h[:])
            idt = cpool.tile([128, 128], F32R, tag="idt")
            nc.sync.dma_start(idt[:], idt_h[:])
            wih_t = cpool.tile([128, 2 * 3 * H], F32R, tag="wih")
            whh_t = cpool.tile([128, 2 * 3 * H], gdt, tag="whh")
            for k in range(2):
                nc.sync.dma_start(
                    wih_t[:, k * 3 * H : (k + 1) * 3 * H],
                    wih_h[k * 128 : (k + 1) * 128, :],
                )
                nc.sync.dma_start(
                    whh_t[:, k * 3 * H : (k + 1) * 3 * H],
                    whh_h[k * 128 : (k + 1) * 128, :],
                )
            wc_t = cpool.tile([128, 2 * H], gdt, tag="wc")
            wmu_t = cpool.tile([128, 2 * C], gdt, tag="wmu")
            for k in range(2):
                nc.sync.dma_start(
                    wc_t[:, k * H : (k + 1) * H], wc_h[k * 128 : (k + 1) * 128, :]
                )
                nc.sync.dma_start(
                    wmu_t[:, k * C : (k + 1) * C], wmu_h[k * 128 : (k + 1) * 128, :]
                )

            def pp(col):  # per-partition operand column
                return cvt[:, col : col + 1]

            # ---- stage 1: x1 = prelu(bn(x @ W_lin.T)), gi = x1 @ W_ih.T ----
            gi_tiles = []
            with tc.tile_pool(name="stage1", bufs=1) as xp:
                xt = xp.tile([128, 5 * BC], F32R, tag="xT")
                for k in range(5):
                    nc.sync.dma_start(
                        xt[:, k * BC : (k + 1) * BC], xT_h[k * 128 : (k + 1) * 128, :]
                    )
                wlin_t = xp.tile([128, 5 * H], F32R, tag="wlin")
                for k in range(5):
                    nc.sync.dma_start(
                        wlin_t[:, k * H : (k + 1) * H], wlin_h[k * 128 : (k + 1) * 128, :]
                    )
                for c in range(NCH):
                    ps_x1 = ps2.tile([128, 2 * CB], F32, tag="g")
                    for k in range(5):
                        for m in range(2):
                            nc.tensor.matmul(
                                ps_x1[:, m * CB : (m + 1) * CB],
                                wlin_t[:, k * H + m * 128 : k * H + (m + 1) * 128],
                                xt[:, k * BC + c * CB : k * BC + (c + 1) * CB],
                                start=(k == 0),
                                stop=(k == 4),
                            )
                    x1p = xp.tile([128, 2 * CB], F32R, tag=f"x1p{c}")
                    for m in range(2):
                        nc.scalar.activation(
                            x1p[:, m * CB : (m + 1) * CB],
                            ps_x1[:, m * CB : (m + 1) * CB],
                            AF.Prelu,
                            bias=pp(CV_T1 + m),
                            scale=pp(CV_S1 + m),
                            alpha=pp(CV_A1),
                        )
                    gi = gip.tile([128, 6 * CB], F32R, tag=f"gi{c}")
                    gi_tiles.append(gi)
                    for m in range(6):
                        ps_gi = ps1.tile([128, CB], F32, tag="p")
                        for k in range(2):
                            nc.tensor.matmul(
                                ps_gi[:],
                                wih_t[:, k * 3 * H + m * 128 : k * 3 * H + (m + 1) * 128],
                                x1p[:, k * CB : (k + 1) * CB],
                                start=(k == 0),
                                stop=(k == 1),
                            )
                        # gi' = gi + (b_ih [+ b_hh for r,z]) folded via bias
                        nc.scalar.activation(
                            gi[:, m * CB : (m + 1) * CB],
                            ps_gi[:],
                            AF.Identity,
                            bias=pp(CV_GIB + m),
                        )

            # ---- GRU + stages 2..4; chunks interleaved per step so their
            # independent dependency chains overlap across engines ----
            gin_bf = []
            hs = []
            packs = [None] * NCH
            for c in range(NCH):
                g_bf = gip.tile([128, 2 * CB], gdt, tag=f"ginb{c}", name=f"ginb{c}")
                nc.scalar.activation(g_bf[:], gi_tiles[c][:, 4 * CB : 6 * CB], AF.Copy)
                gin_bf.append(g_bf)
                h = hp.tile([128, 2 * CB], gdt, tag=f"h{c}")
                nc.vector.memset(h[:], 0.0)
                hs.append(h)
            w2s = [None] * NCH

            def emit_stages(t, c):
                """stages 2..4 for step t of chunk c (conv + prelu3 + W_mu head).
                Emitted one step late so this ready work fills the engines
                while step t+1's serial gate chain runs."""
                ps_cv = ps2.tile([128, 2 * CB], F32, tag="g", name=f"pscv{c}")
                w2 = w2s[c]
                for k in range(2):
                    for m in range(2):
                        nc.tensor.matmul(
                            ps_cv[:, m * CB : (m + 1) * CB],
                            wc_t[:, k * H + m * 128 : k * H + (m + 1) * 128],
                            w2[:, k * CB : (k + 1) * CB],
                            start=(k == 0),
                            stop=(k == 1),
                        )
                y2 = gp.tile([128, 2 * CB], gdt, tag="y2", name=f"y2_{c}")
                for m in range(2):
                    nc.scalar.activation(
                        y2[:, m * CB : (m + 1) * CB],
                        ps_cv[:, m * CB : (m + 1) * CB],
                        AF.Prelu,
                        bias=pp(CV_T3 + m),
                        scale=pp(CV_S3 + m),
                        alpha=pp(CV_A3),
                    )
                j = t % 4
                if j == 0:
                    packs[c] = ps1.tile([128, CB], F32, tag="p", name=f"pack{c}")
                ps_pack = packs[c]
                for k in range(2):
                    nc.tensor.matmul(
                        ps_pack[32 * j : 32 * (j + 1), :],
                        wmu_t[:, k * C : (k + 1) * C],
                        y2[:, k * CB : (k + 1) * CB],
                        start=(k == 0),
                        stop=(k == 1),
                        tile_position=(0, 32 * j),
                    )
                if j == 3:
                    pk = sp.tile([128, CB], F32, tag="pk", name=f"pk{c}")
                    nc.scalar.activation(pk[:], ps_pack[:], AF.Identity, bias=pp(CV_BMU))
                    tr = sp.tile([128, CB], F32, tag="tr", name=f"tr{c}")
                    nc.vector.transpose(tr[:], pk[:])
                    l4 = t // 4
                    dst = out_h[:].rearrange(
                        "(c bh bl) (l4 li cc) -> c l4 li bl bh cc",
                        c=NCH, bh=CB // 32, bl=32, l4=L // 4, li=4, cc=C,
                    )
                    for li in range(4):
                        nc.sync.dma_start(
                            dst[c, l4, li],
                            tr[32 * li : 32 * (li + 1), :].rearrange(
                                "p (bh cc) -> p bh cc", bh=CB // 32
                            ),
                        )

            for t in range(L):
                # stage work from the previous step first: it is ready now and
                # keeps PE/ACT fed while this step's gate chain serializes
                if t >= 1:
                    for c in range(NCH):
                        emit_stages(t - 1, c)

                psr, psz, psn = [], [], []
                for c in range(NCH):
                    psr.append(ps2.tile([128, 2 * CB], F32, tag="g", name=f"psr{c}"))
                    psz.append(ps2.tile([128, 2 * CB], F32, tag="g", name=f"psz{c}"))
                    psn.append(ps2.tile([128, 2 * CB], F32, tag="g", name=f"psn{c}"))
                # identity preloads inject gi' for the r,z gates
                for c in range(NCH):
                    for g, ps in ((0, psr[c]), (1, psz[c])):
                        for m in range(2):
                            nc.tensor.matmul(
                                ps[:, m * CB : (m + 1) * CB],
                                idt[:],
                                gi_tiles[c][:, (2 * g + m) * CB : (2 * g + m + 1) * CB],
                                start=True,
                                stop=False,
                            )
                for c in range(NCH):
                    for k in range(2):
                        for g, ps in ((0, psr[c]), (1, psz[c]), (2, psn[c])):
                            for m in range(2):
                                row = 2 * g + m
                                nc.tensor.matmul(
                                    ps[:, m * CB : (m + 1) * CB],
                                    whh_t[:, k * 3 * H + row * 128 : k * 3 * H + (row + 1) * 128],
                                    hs[c][:, k * CB : (k + 1) * CB],
                                    start=(g == 2 and k == 0),
                                    stop=(k == 1),
                                )
                # gate chains, op-interleaved across chunks so ACT and DVE
                # always have the other chunk's op available
                r_sb, z_sb, t_sb, a_sb, n_sb, u_sb, v_sb = ({} for _ in range(7))
                for c in range(NCH):
                    r_sb[c] = gp.tile([128, 2 * CB], gdt, tag="r", name=f"r{c}")
                    nc.scalar.activation(r_sb[c][:], psr[c][:], AF.Sigmoid)
                for c in range(NCH):
                    z_sb[c] = gp.tile([128, 2 * CB], gdt, tag="z", name=f"z{c}")
                    nc.scalar.activation(z_sb[c][:], psz[c][:], AF.Sigmoid)
                for c in range(NCH):
                    t_sb[c] = gp.tile([128, 2 * CB], gdt, tag="t", name=f"t{c}")
                    for m in range(2):
                        # (gh_n + b_hh_n) * r
                        nc.vector.scalar_tensor_tensor(
                            t_sb[c][:, m * CB : (m + 1) * CB],
                            psn[c][:, m * CB : (m + 1) * CB],
                            pp(CV_BHN + m),
                            r_sb[c][:, m * CB : (m + 1) * CB],
                            op0=OP.add,
                            op1=OP.mult,
                        )
                for c in range(NCH):
                    a_sb[c] = gp.tile([128, 2 * CB], gdt, tag="a", name=f"a{c}")
                    nc.vector.tensor_tensor(a_sb[c][:], t_sb[c][:], gin_bf[c][:], OP.add)
                for c in range(NCH):
                    n_sb[c] = gp.tile([128, 2 * CB], gdt, tag="n", name=f"n{c}")
                    nc.scalar.activation(n_sb[c][:], a_sb[c][:], AF.Tanh)
                for c in range(NCH):
                    u_sb[c] = gp.tile([128, 2 * CB], gdt, tag="u", name=f"u{c}")
                    nc.vector.tensor_tensor(u_sb[c][:], hs[c][:], n_sb[c][:], OP.subtract)
                for c in range(NCH):
                    v_sb[c] = gp.tile([128, 2 * CB], gdt, tag="v", name=f"v{c}")
                    nc.vector.tensor_tensor(v_sb[c][:], z_sb[c][:], u_sb[c][:], OP.mult)
                for c in range(NCH):
                    # h' = n + z*(h - n)
                    h = hp.tile([128, 2 * CB], gdt, tag=f"h{c}", name=f"h{c}")
                    nc.vector.tensor_tensor(h[:], n_sb[c][:], v_sb[c][:], OP.add)
                    hs[c] = h
                for c in range(NCH):
                    # stage 2: w2 = prelu(bn2(h_t)) on DVE; consumed next period
                    w2a = gp.tile([128, 2 * CB], gdt, tag="w2a", name=f"w2a{c}")
                    for m in range(2):
                        nc.vector.tensor_scalar(
                            w2a[:, m * CB : (m + 1) * CB],
                            hs[c][:, m * CB : (m + 1) * CB],
                            pp(CV_S2 + m),
                            pp(CV_T2 + m),
                            op0=OP.mult,
                            op1=OP.add,
                        )
                    w2 = gp.tile([128, 2 * CB], gdt, tag="w2", name=f"w2_{c}")
                    # prelu(x) = max(a*x, x) for 0 <= a <= 1
                    nc.vector.scalar_tensor_tensor(
                        w2[:], w2a[:], pp(CV_A2), w2a[:], op0=OP.mult, op1=OP.max
                    )
                    w2s[c] = w2
            for c in range(NCH):
                emit_stages(L - 1, c)

    nc.compile()
    return nc


def _prep_inputs(inputs):
    f32 = np.float32
    gnp = _np_gdt()
    x = np.ascontiguousarray(np.asarray(inputs["h_w_action"], f32).reshape(B, F))
    W_lin = np.asarray(inputs["W_lin"], f32)
    b_lin = np.asarray(inputs["b_lin"], f32)
    W_ih = np.asarray(inputs["W_ih"], f32)
    W_hh = np.asarray(inputs["W_hh"], f32)
    b_ih = np.asarray(inputs["b_ih"], f32)
    b_hh = np.asarray(inputs["b_hh"], f32)
    Wc = np.asarray(inputs["Wc"], f32)
    bc = np.asarray(inputs["bc"], f32)
    W_mu = np.asarray(inputs["W_mu"], f32)
    b_mu = np.asarray(inputs["b_mu"], f32)

    def bnfold(g, beta, m, v):
        s = g / np.sqrt(v + EPS)
        return s, beta - m * s

    s1, t1 = bnfold(inputs["g1"], inputs["beta1"], inputs["m1"], inputs["v1"])
    s2, t2 = bnfold(inputs["g2"], inputs["beta2"], inputs["m2"], inputs["v2"])
    s3, t3 = bnfold(inputs["g3"], inputs["beta3"], inputs["m3"], inputs["v3"])
    s1, t1, s2, t2, s3, t3 = (np.asarray(a, f32) for a in (s1, t1, s2, t2, s3, t3))
    t1 = t1 + s1 * b_lin          # fold linear bias into bn1 shift
    t3 = t3 + s3 * bc             # fold conv bias into bn3 shift
    gib = b_ih.copy()
    gib[: 2 * H] += b_hh[: 2 * H]  # fold b_hh into gi for the r,z gates
    bhn = b_hh[2 * H :]

    cv = np.zeros((128, NV), f32)
    for col, vec in ((CV_S1, s1), (CV_T1, t1), (CV_S2, s2), (CV_T2, t2),
                     (CV_S3, s3), (CV_T3, t3)):
        cv[:, col] = vec[:128]
        cv[:, col + 1] = vec[128:]
    for m in range(6):
        cv[:, CV_GIB + m] = gib[m * 128 : (m + 1) * 128]
    cv[:, CV_BHN] = bhn[:128]
    cv[:, CV_BHN + 1] = bhn[128:]
    cv[:, CV_BMU] = np.tile(b_mu, 4)
    cv[:, CV_A1] = f32(np.asarray(inputs["a1"]).reshape(-1)[0])
    cv[:, CV_A2] = f32(np.asarray(inputs["a2"]).reshape(-1)[0])
    cv[:, CV_A3] = f32(np.asarray(inputs["a3"]).reshape(-1)[0])

    shared = {
        "wlin": np.ascontiguousarray(W_lin.T),
        "wih": np.ascontiguousarray(W_ih.T),
        "whh": np.ascontiguousarray(W_hh.T).astype(gnp),
        "wc": np.ascontiguousarray(Wc.T).astype(gnp),
        "wmu": np.ascontiguousarray(W_mu.T).astype(gnp),
        "cv": cv,
        "idt": np.eye(128, dtype=f32),
    }
    in_maps = []
    for i in range(NCORES):
        m = dict(shared)
        m["xT"] = np.ascontiguousarray(x[i * BC : (i + 1) * BC, :].T)
        in_maps.append(m)
    return in_maps


def kernel(**inputs) -> np.ndarray:
    if "nc" not in _CACHE:
        _CACHE["nc"] = build_program()
    nc = _CACHE["nc"]
    in_maps = _prep_inputs(inputs)
    res = bass_utils.run_bass_kernel_spmd(nc, in_maps, core_ids=list(range(NCORES)))
    outs = [np.asarray(r["out"], np.float32) for r in res.results]
    return np.concatenate(outs, axis=0).reshape(E, S, L, C)


# revision 12
# speedup vs baseline: 1.7258x; 1.2325x over previous
"""Trainium2 Bass kernel for nn_Comm_OUT (Linear+BN+PReLU -> 20-step GRU ->
BN+PReLU -> 1x1 conv -> BN+PReLU -> linear head), data-parallel over 8 cores.

Layout strategy: everything on-chip is kept "transposed" (channels on SBUF
partitions, batch on the free dim) so the GRU recurrence never needs an
on-chip transpose:
    gh.T[3H, B] = W_hh @ h.T   (W_hh.T tiles are the stationary operand)
The per-step `gi + gh` adds are folded into the PE via identity-matmul PSUM
preloads, biases are folded into per-partition activation bias operands, and
BN+PReLU collapses into single Prelu activations.
"""

import numpy as np
import ml_dtypes

import concourse.bacc as bacc
import concourse.mybir as mybir
import concourse.tile as tile
from concourse import bass_utils

AF = mybir.ActivationFunctionType
OP = mybir.AluOpType
F32 = mybir.dt.float32
F32R = mybir.dt.float32r
BF16 = mybir.dt.bfloat16

E, S, F, H, C, L = 64, 128, 640, 256, 32, 20
EPS = 1e-5
NCORES = 8
B = E * S              # 8192
BC = B // NCORES       # 1024 batch rows per core
NCH = 2                # chunks per core
CB = BC // NCH         # 512 batch rows per chunk (PSUM-bank friendly)

# --- precision config -------------------------------------------------------
# GATE_BF16: store gates/hidden state (and conv/W_mu operands) in bf16 for 2x
# DVE tensor_tensor throughput. Matmuls otherwise run fp32 bitcast to fp32r
# (full PE speed at N>=256).
GATE_BF16 = True

# const-vector column indices (packed [128, NV] tensor, one column per
# per-partition operand vector)
CV_S1, CV_T1, CV_GIB, CV_BHN, CV_S2, CV_T2, CV_S3, CV_T3 = 0, 2, 4, 10, 12, 14, 16, 18
CV_BMU, CV_A1, CV_A2, CV_A3 = 20, 21, 22, 23
NV = 24

_CACHE: dict = {}


def _gdt():
    return BF16 if GATE_BF16 else F32


def _np_gdt():
    return ml_dtypes.bfloat16 if GATE_BF16 else np.float32


def _mm(x):
    """bitcast fp32 APs to fp32r for full-rate PE; bf16 passes through."""
    if x.dtype == F32:
        return x.bitcast(F32R)
    return x


def build_program():
    gdt = _gdt()
    nc = bacc.Bacc("TRN2", target_bir_lowering=False, debug=False)

    xT_h = nc.dram_tensor("xT", [F, BC], F32R, kind="ExternalInput")
    wlin_h = nc.dram_tensor("wlin", [F, H], F32R, kind="ExternalInput")
    wih_h = nc.dram_tensor("wih", [H, 3 * H], F32R, kind="ExternalInput")
    whh_h = nc.dram_tensor("whh", [H, 3 * H], gdt, kind="ExternalInput")
    wc_h = nc.dram_tensor("wc", [H, H], gdt, kind="ExternalInput")
    wmu_h = nc.dram_tensor("wmu", [H, C], gdt, kind="ExternalInput")
    cv_h = nc.dram_tensor("cv", [128, NV], F32, kind="ExternalInput")
    idt_h = nc.dram_tensor("idt", [128, 128], F32R, kind="ExternalInput")
    out_h = nc.dram_tensor("out", [BC, L * C], F32, kind="ExternalOutput")

    with tile.TileContext(nc) as tc:
        with (
            tc.tile_pool(name="consts", bufs=1) as cpool,
            tc.tile_pool(name="gi", bufs=1) as gip,
            tc.tile_pool(name="hp", bufs=2) as hp,
            tc.tile_pool(name="gates", bufs=3) as gp,
            tc.tile_pool(name="s24", bufs=2) as sp,
            tc.tile_pool(name="ps2", bufs=3, space="PSUM") as ps2,
            tc.tile_pool(name="ps1", bufs=2, space="PSUM") as ps1,
        ):
            cvt = cpool.tile([128, NV], F32, tag="cv")
            nc.sync.dma_start(cvt[:], cv_h[:])
            idt = cpool.tile([128, 128], F32R, tag="idt")
            nc.sync.dma_start(idt[:], idt_h[:])
            wih_t = cpool.tile([128, 2 * 3 * H], F32R, tag="wih")
            whh_t = cpool.tile([128, 2 * 3 * H], gdt, tag="whh")
            for k in range(2):
                nc.sync.dma_start(
                    wih_t[:, k * 3 * H : (k + 1) * 3 * H],
                    wih_h[k * 128 : (k + 1) * 128, :],
                )
                nc.sync.dma_start(
                    whh_t[:, k * 3 * H : (k + 1) * 3 * H],
                    whh_h[k * 128 : (k + 1) * 128, :],
                )
            wc_t = cpool.tile([128, 2 * H], gdt, tag="wc")
            wmu_t = cpool.tile([128, 2 * C], gdt, tag="wmu")
            for k in range(2):
                nc.sync.dma_start(
                    wc_t[:, k * H : (k + 1) * H], wc_h[k * 128 : (k + 1) * 128, :]
                )
                nc.sync.dma_start(
                    wmu_t[:, k * C : (k + 1) * C], wmu_h[k * 128 : (k + 1) * 128, :]
                )

            def pp(col):  # per-partition operand column
                return cvt[:, col : col + 1]

            # ---- stage 1: x1 = prelu(bn(x @ W_lin.T)), gi = x1 @ W_ih.T ----
            gi_tiles = []
            with tc.tile_pool(name="stage1", bufs=1) as xp:
                xt = xp.tile([128, 5 * BC], F32R, tag="xT")
                for k in range(5):
                    nc.sync.dma_start(
                        xt[:, k * BC : (k + 1) * BC], xT_h[k * 128 : (k + 1) * 128, :]
                    )
                wlin_t = xp.tile([128, 5 * H], F32R, tag="wlin")
                for k in range(5):
                    nc.sync.dma_start(
                        wlin_t[:, k * H : (k + 1) * H], wlin_h[k * 128 : (k + 1) * 128, :]
                    )
                for c in range(NCH):
                    ps_x1 = ps2.tile([128, 2 * CB], F32, tag="g")
                    for k in range(5):
                        for m in range(2):
                            nc.tensor.matmul(
                                ps_x1[:, m * CB : (m + 1) * CB],
                                wlin_t[:, k * H + m * 128 : k * H + (m + 1) * 128],
                                xt[:, k * BC + c * CB : k * BC + (c + 1) * CB],
                                start=(k == 0),
                                stop=(k == 4),
                            )
                    x1p = xp.tile([128, 2 * CB], F32R, tag=f"x1p{c}")
                    for m in range(2):
                        nc.scalar.activation(
                            x1p[:, m * CB : (m + 1) * CB],
                            ps_x1[:, m * CB : (m + 1) * CB],
                            AF.Prelu,
                            bias=pp(CV_T1 + m),
                            scale=pp(CV_S1 + m),
                            alpha=pp(CV_A1),
                        )
                    gi = gip.tile([128, 6 * CB], F32R, tag=f"gi{c}")
                    gi_tiles.append(gi)
                    for m in range(6):
                        ps_gi = ps1.tile([128, CB], F32, tag="p")
                        for k in range(2):
                            nc.tensor.matmul(
                                ps_gi[:],
                                wih_t[:, k * 3 * H + m * 128 : k * 3 * H + (m + 1) * 128],
                                x1p[:, k * CB : (k + 1) * CB],
                                start=(k == 0),
                                stop=(k == 1),
                            )
                        # gi' = gi + (b_ih [+ b_hh for r,z]) folded via bias
                        nc.scalar.activation(
                            gi[:, m * CB : (m + 1) * CB],
                            ps_gi[:],
                            AF.Identity,
                            bias=pp(CV_GIB + m),
                        )

            # ---- GRU + stages 2..4; chunks interleaved per step so their
            # independent dependency chains overlap across engines ----
            gin_bf = []
            hs = []
            packs = [None] * NCH
            for c in range(NCH):
                g_bf = gip.tile([128, 2 * CB], gdt, tag=f"ginb{c}", name=f"ginb{c}")
                nc.scalar.activation(g_bf[:], gi_tiles[c][:, 4 * CB : 6 * CB], AF.Copy)
                gin_bf.append(g_bf)
                h = hp.tile([128, 2 * CB], gdt, tag=f"h{c}")
                nc.vector.memset(h[:], 0.0)
                hs.append(h)
            w2s = [None] * NCH

            def emit_stages(t, c):
                """stages 2..4 for step t of chunk c (conv + prelu3 + W_mu head).
                Emitted one step late so this ready work fills the engines
                while step t+1's serial gate chain runs."""
                ps_cv = ps2.tile([128, 2 * CB], F32, tag="g", name=f"pscv{c}")
                w2 = w2s[c]
                for k in range(2):
                    for m in range(2):
                        nc.tensor.matmul(
                            ps_cv[:, m * CB : (m + 1) * CB],
                            wc_t[:, k * H + m * 128 : k * H + (m + 1) * 128],
                            w2[:, k * CB : (k + 1) * CB],
                            start=(k == 0),
                            stop=(k == 1),
                        )
                y2 = gp.tile([128, 2 * CB], gdt, tag="y2", name=f"y2_{c}")
                for m in range(2):
                    nc.scalar.activation(
                        y2[:, m * CB : (m + 1) * CB],
                        ps_cv[:, m * CB : (m + 1) * CB],
                        AF.Prelu,
                        bias=pp(CV_T3 + m),
                        scale=pp(CV_S3 + m),
                        alpha=pp(CV_A3),
                    )
                j = t % 4
                if j == 0:
                    packs[c] = ps1.tile([128, CB], F32, tag="p", name=f"pack{c}")
                ps_pack = packs[c]
                for k in range(2):
                    nc.tensor.matmul(
                        ps_pack[32 * j : 32 * (j + 1), :],
                        wmu_t[:, k * C : (k + 1) * C],
                        y2[:, k * CB : (k + 1) * CB],
                        start=(k == 0),
                        stop=(k == 1),
                        tile_position=(0, 32 * j),
                    )
                if j == 3:
                    pk = sp.tile([128, CB], F32, tag="pk", name=f"pk{c}")
                    nc.scalar.activation(pk[:], ps_pack[:], AF.Identity, bias=pp(CV_BMU))
                    tr = sp.tile([128, CB], F32, tag="tr", name=f"tr{c}")
                    nc.vector.transpose(tr[:], pk[:])
                    l4 = t // 4
                    dst = out_h[:].rearrange(
                        "(c bh bl) (l4 li cc) -> c l4 li bl bh cc",
                        c=NCH, bh=CB // 32, bl=32, l4=L // 4, li=4, cc=C,
                    )
                    for li in range(4):
                        nc.sync.dma_start(
                            dst[c, l4, li],
                            tr[32 * li : 32 * (li + 1), :].rearrange(
                                "p (bh cc) -> p bh cc", bh=CB // 32
                            ),
                        )

            def emit_gate_mms(g, psl):
                # preloads then real matmuls for one gate, both chunks; gate
                # order r,z,n lets the ACT/DVE chain start while n still runs
                for c in range(NCH):
                    if g < 2:
                        for m in range(2):
                            nc.tensor.matmul(
                                psl[c][:, m * CB : (m + 1) * CB],
                                idt[:],
                                gi_tiles[c][:, (2 * g + m) * CB : (2 * g + m + 1) * CB],
                                start=True,
                                stop=False,
                            )
                for c in range(NCH):
                    for k in range(2):
                        for m in range(2):
                            row = 2 * g + m
                            nc.tensor.matmul(
                                psl[c][:, m * CB : (m + 1) * CB],
                                whh_t[:, k * 3 * H + row * 128 : k * 3 * H + (row + 1) * 128],
                                hs[c][:, k * CB : (k + 1) * CB],
                                start=(g == 2 and k == 0),
                                stop=(k == 1),
                            )

            for t in range(L):
                psr, psz, psn = [], [], []
                for c in range(NCH):
                    psr.append(ps2.tile([128, 2 * CB], F32, tag="g", name=f"psr{c}"))
                    psz.append(ps2.tile([128, 2 * CB], F32, tag="g", name=f"psz{c}"))
                    psn.append(ps2.tile([128, 2 * CB], F32, tag="g", name=f"psn{c}"))
                emit_gate_mms(0, psr)
                emit_gate_mms(1, psz)
                emit_gate_mms(2, psn)
                # previous step's stage work now: ready, fills PE/ACT/DVE tails
                # while this step's gate chain serializes
                if t >= 1:
                    for c in range(NCH):
                        emit_stages(t - 1, c)
                # gate chains, op-interleaved across chunks so ACT and DVE
                # always have the other chunk's op available
                r_sb, z_sb, t_sb, a_sb, n_sb, u_sb, v_sb = ({} for _ in range(7))
                for c in range(NCH):
                    r_sb[c] = gp.tile([128, 2 * CB], gdt, tag="r", name=f"r{c}")
                    nc.scalar.activation(r_sb[c][:], psr[c][:], AF.Sigmoid)
                for c in range(NCH):
                    z_sb[c] = gp.tile([128, 2 * CB], gdt, tag="z", name=f"z{c}")
                    nc.scalar.activation(z_sb[c][:], psz[c][:], AF.Sigmoid)
                for c in range(NCH):
                    t_sb[c] = gp.tile([128, 2 * CB], gdt, tag="t", name=f"t{c}")
                    for m in range(2):
                        # (gh_n + b_hh_n) * r
                        nc.vector.scalar_tensor_tensor(
                            t_sb[c][:, m * CB : (m + 1) * CB],
                            psn[c][:, m * CB : (m + 1) * CB],
                            pp(CV_BHN + m),
                            r_sb[c][:, m * CB : (m + 1) * CB],
                            op0=OP.add,
                            op1=OP.mult,
                        )
                for c in range(NCH):
                    a_sb[c] = gp.tile([128, 2 * CB], gdt, tag="a", name=f"a{c}")
                    nc.vector.tensor_tensor(a_sb[c][:], t_sb[c][:], gin_bf[c][:], OP.add)
                for c in range(NCH):
                    n_sb[c] = gp.tile([128, 2 * CB], gdt, tag="n", name=f"n{c}")
                    nc.scalar.activation(n_sb[c][:], a_sb[c][:], AF.Tanh)
                for c in range(NCH):
                    u_sb[c] = gp.tile([128, 2 * CB], gdt, tag="u", name=f"u{c}")
                    nc.vector.tensor_tensor(u_sb[c][:], hs[c][:], n_sb[c][:], OP.subtract)
                for c in range(NCH):
                    v_sb[c] = gp.tile([128, 2 * CB], gdt, tag="v", name=f"v{c}")
                    nc.vector.tensor_tensor(v_sb[c][:], z_sb[c][:], u_sb[c][:], OP.mult)
                for c in range(NCH):
                    # h' = n + z*(h - n)
                    h = hp.tile([128, 2 * CB], gdt, tag=f"h{c}", name=f"h{c}")
                    nc.vector.tensor_tensor(h[:], n_sb[c][:], v_sb[c][:], OP.add)
                    hs[c] = h
                for c in range(NCH):
                    # stage 2: w2 = prelu(bn2(h_t)) on DVE; consumed next period
                    w2a = gp.tile([128, 2 * CB], gdt, tag="w2a", name=f"w2a{c}")
                    for m in range(2):
                        nc.vector.tensor_scalar(
                            w2a[:, m * CB : (m + 1) * CB],
                            hs[c][:, m * CB : (m + 1) * CB],
                            pp(CV_S2 + m),
                            pp(CV_T2 + m),
                            op0=OP.mult,
                            op1=OP.add,
                        )
                    w2 = gp.tile([128, 2 * CB], gdt, tag="w2", name=f"w2_{c}")
                    # prelu(x) = max(a*x, x) for 0 <= a <= 1
                    nc.vector.scalar_tensor_tensor(
                        w2[:], w2a[:], pp(CV_A2), w2a[:], op0=OP.mult, op1=OP.max
                    )
                    w2s[c] = w2
            for c in range(NCH):
                emit_stages(L - 1, c)

    nc.compile()
    return nc


def _prep_inputs(inputs):
    f32 = np.float32
    gnp = _np_gdt()
    x = np.ascontiguousarray(np.asarray(inputs["h_w_action"], f32).reshape(B, F))
    W_lin = np.asarray(inputs["W_lin"], f32)
    b_lin = np.asarray(inputs["b_lin"], f32)
    W_ih = np.asarray(inputs["W_ih"], f32)
    W_hh = np.asarray(inputs["W_hh"], f32)
    b_ih = np.asarray(inputs["b_ih"], f32)
    b_hh = np.asarray(inputs["b_hh"], f32)
    Wc = np.asarray(inputs["Wc"], f32)
    bc = np.asarray(inputs["bc"], f32)
    W_mu = np.asarray(inputs["W_mu"], f32)
    b_mu = np.asarray(inputs["b_mu"], f32)

    def bnfold(g, beta, m, v):
        s = g / np.sqrt(v + EPS)
        return s, beta - m * s

    s1, t1 = bnfold(inputs["g1"], inputs["beta1"], inputs["m1"], inputs["v1"])
    s2, t2 = bnfold(inputs["g2"], inputs["beta2"], inputs["m2"], inputs["v2"])
    s3, t3 = bnfold(inputs["g3"], inputs["beta3"], inputs["m3"], inputs["v3"])
    s1, t1, s2, t2, s3, t3 = (np.asarray(a, f32) for a in (s1, t1, s2, t2, s3, t3))
    t1 = t1 + s1 * b_lin          # fold linear bias into bn1 shift
    t3 = t3 + s3 * bc             # fold conv bias into bn3 shift
    gib = b_ih.copy()
    gib[: 2 * H] += b_hh[: 2 * H]  # fold b_hh into gi for the r,z gates
    bhn = b_hh[2 * H :]

    cv = np.zeros((128, NV), f32)
    for col, vec in ((CV_S1, s1), (CV_T1, t1), (CV_S2, s2), (CV_T2, t2),
                     (CV_S3, s3), (CV_T3, t3)):
        cv[:, col] = vec[:128]
        cv[:, col + 1] = vec[128:]
    for m in range(6):
        cv[:, CV_GIB + m] = gib[m * 128 : (m + 1) * 128]
    cv[:, CV_BHN] = bhn[:128]
    cv[:, CV_BHN + 1] = bhn[128:]
    cv[:, CV_BMU] = np.tile(b_mu, 4)
    cv[:, CV_A1] = f32(np.asarray(inputs["a1"]).reshape(-1)[0])
    cv[:, CV_A2] = f32(np.asarray(inputs["a2"]).reshape(-1)[0])
    cv[:, CV_A3] = f32(np.asarray(inputs["a3"]).reshape(-1)[0])

    shared = {
        "wlin": np.ascontiguousarray(W_lin.T),
        "wih": np.ascontiguousarray(W_ih.T),
        "whh": np.ascontiguousarray(W_hh.T).astype(gnp),
        "wc": np.ascontiguousarray(Wc.T).astype(gnp),
        "wmu": np.ascontiguousarray(W_mu.T).astype(gnp),
        "cv": cv,
        "idt": np.eye(128, dtype=f32),
    }
    in_maps = []
    for i in range(NCORES):
        m = dict(shared)
        m["xT"] = np.ascontiguousarray(x[i * BC : (i + 1) * BC, :].T)
        in_maps.append(m)
    return in_maps


def kernel(**inputs) -> np.ndarray:
    if "nc" not in _CACHE:
        _CACHE["nc"] = build_program()
    nc = _CACHE["nc"]
    in_maps = _prep_inputs(inputs)
    res = bass_utils.run_bass_kernel_spmd(nc, in_maps, core_ids=list(range(NCORES)))
    outs = [np.asarray(r["out"], np.float32) for r in res.results]
    return np.concatenate(outs, axis=0).reshape(E, S, L, C)


# revision 16
# speedup vs baseline: 2.0319x; 1.1774x over previous
"""Trainium2 Bass kernel for nn_Comm_OUT (Linear+BN+PReLU -> 20-step GRU ->
BN+PReLU -> 1x1 conv -> BN+PReLU -> linear head), data-parallel over 8 cores.

Layout strategy: everything on-chip is kept "transposed" (channels on SBUF
partitions, batch on the free dim) so the GRU recurrence never needs an
on-chip transpose:
    gh.T[3H, B] = W_hh @ h.T   (W_hh.T tiles are the stationary operand)
The per-step `gi + gh` adds are folded into the PE via identity-matmul PSUM
preloads, biases are folded into per-partition activation bias operands, and
BN+PReLU collapses into single Prelu activations.
"""

import numpy as np
import ml_dtypes

import concourse.bacc as bacc
import concourse.mybir as mybir
import concourse.tile as tile
from concourse import bass_utils

AF = mybir.ActivationFunctionType
OP = mybir.AluOpType
F32 = mybir.dt.float32
F32R = mybir.dt.float32r
BF16 = mybir.dt.bfloat16

E, S, F, H, C, L = 64, 128, 640, 256, 32, 20
EPS = 1e-5
NCORES = 8
B = E * S              # 8192
BC = B // NCORES       # 1024 batch rows per core
NCH = 2                # chunks per core
CB = BC // NCH         # 512 batch rows per chunk (PSUM-bank friendly)

# --- precision config -------------------------------------------------------
# GATE_BF16: store gates/hidden state (and conv/W_mu operands) in bf16 for 2x
# DVE tensor_tensor throughput. Matmuls otherwise run fp32 bitcast to fp32r
# (full PE speed at N>=256).
GATE_BF16 = True

# const-vector column indices (packed [128, NV] tensor, one column per
# per-partition operand vector)
CV_S1, CV_T1, CV_GIB, CV_BHN, CV_S2, CV_T2, CV_S3, CV_T3 = 0, 2, 4, 10, 12, 14, 16, 18
CV_BMU, CV_A1, CV_A2, CV_A3 = 20, 21, 22, 23
NV = 24

_CACHE: dict = {}


def _gdt():
    return BF16 if GATE_BF16 else F32


def _np_gdt():
    return ml_dtypes.bfloat16 if GATE_BF16 else np.float32


def _mm(x):
    """bitcast fp32 APs to fp32r for full-rate PE; bf16 passes through."""
    if x.dtype == F32:
        return x.bitcast(F32R)
    return x


def build_program(bhn_zero=True):
    BHN_ZERO = bhn_zero
    gdt = _gdt()
    nc = bacc.Bacc("TRN2", target_bir_lowering=False, debug=False)

    xT_h = nc.dram_tensor("xT", [F, BC], F32R, kind="ExternalInput")
    wlin_h = nc.dram_tensor("wlin", [F, H], F32R, kind="ExternalInput")
    wih_h = nc.dram_tensor("wih", [H, 3 * H], F32R, kind="ExternalInput")
    whh_h = nc.dram_tensor("whh", [H, 3 * H], gdt, kind="ExternalInput")
    wc_h = nc.dram_tensor("wc", [H, H], gdt, kind="ExternalInput")
    wmu_h = nc.dram_tensor("wmu", [H, C], gdt, kind="ExternalInput")
    cv_h = nc.dram_tensor("cv", [128, NV], F32, kind="ExternalInput")
    idt_h = nc.dram_tensor("idt", [128, 128], F32R, kind="ExternalInput")
    out_h = nc.dram_tensor("out", [BC, L * C], F32, kind="ExternalOutput")

    with tile.TileContext(nc) as tc:
        with (
            tc.tile_pool(name="consts", bufs=1) as cpool,
            tc.tile_pool(name="gi", bufs=1) as gip,
            tc.tile_pool(name="hp", bufs=2) as hp,
            tc.tile_pool(name="gates", bufs=3) as gp,
            tc.tile_pool(name="s24", bufs=2) as sp,
            tc.tile_pool(name="ps2", bufs=3, space="PSUM") as ps2,
            tc.tile_pool(name="ps1", bufs=2, space="PSUM") as ps1,
        ):
            cvt = cpool.tile([128, NV], F32, tag="cv")
            nc.sync.dma_start(cvt[:], cv_h[:])
            idt = cpool.tile([128, 128], F32R, tag="idt")
            nc.sync.dma_start(idt[:], idt_h[:])
            wih_t = cpool.tile([128, 2 * 3 * H], F32R, tag="wih")
            whh_t = cpool.tile([128, 2 * 3 * H], gdt, tag="whh")
            for k in range(2):
                nc.sync.dma_start(
                    wih_t[:, k * 3 * H : (k + 1) * 3 * H],
                    wih_h[k * 128 : (k + 1) * 128, :],
                )
                nc.sync.dma_start(
                    whh_t[:, k * 3 * H : (k + 1) * 3 * H],
                    whh_h[k * 128 : (k + 1) * 128, :],
                )
            wc_t = cpool.tile([128, 2 * H], gdt, tag="wc")
            wmu_t = cpool.tile([128, 2 * C], gdt, tag="wmu")
            for k in range(2):
                nc.sync.dma_start(
                    wc_t[:, k * H : (k + 1) * H], wc_h[k * 128 : (k + 1) * 128, :]
                )
                nc.sync.dma_start(
                    wmu_t[:, k * C : (k + 1) * C], wmu_h[k * 128 : (k + 1) * 128, :]
                )

            def pp(col):  # per-partition operand column
                return cvt[:, col : col + 1]

            # ---- stage 1: x1 = prelu(bn(x @ W_lin.T)), gi = x1 @ W_ih.T ----
            gi_tiles = []
            with tc.tile_pool(name="stage1", bufs=1) as xp:
                xt = xp.tile([128, 5 * BC], F32R, tag="xT")
                for k in range(5):
                    nc.sync.dma_start(
                        xt[:, k * BC : (k + 1) * BC], xT_h[k * 128 : (k + 1) * 128, :]
                    )
                wlin_t = xp.tile([128, 5 * H], F32R, tag="wlin")
                for k in range(5):
                    nc.sync.dma_start(
                        wlin_t[:, k * H : (k + 1) * H], wlin_h[k * 128 : (k + 1) * 128, :]
                    )
                for c in range(NCH):
                    ps_x1 = ps2.tile([128, 2 * CB], F32, tag="g")
                    for k in range(5):
                        for m in range(2):
                            nc.tensor.matmul(
                                ps_x1[:, m * CB : (m + 1) * CB],
                                wlin_t[:, k * H + m * 128 : k * H + (m + 1) * 128],
                                xt[:, k * BC + c * CB : k * BC + (c + 1) * CB],
                                start=(k == 0),
                                stop=(k == 4),
                            )
                    x1p = xp.tile([128, 2 * CB], F32R, tag=f"x1p{c}")
                    for m in range(2):
                        nc.scalar.activation(
                            x1p[:, m * CB : (m + 1) * CB],
                            ps_x1[:, m * CB : (m + 1) * CB],
                            AF.Prelu,
                            bias=pp(CV_T1 + m),
                            scale=pp(CV_S1 + m),
                            alpha=pp(CV_A1),
                        )
                    gi = gip.tile([128, 6 * CB], F32R, tag=f"gi{c}")
                    gi_tiles.append(gi)
                    for m in range(6):
                        ps_gi = ps1.tile([128, CB], F32, tag="p")
                        for k in range(2):
                            nc.tensor.matmul(
                                ps_gi[:],
                                wih_t[:, k * 3 * H + m * 128 : k * 3 * H + (m + 1) * 128],
                                x1p[:, k * CB : (k + 1) * CB],
                                start=(k == 0),
                                stop=(k == 1),
                            )
                        # gi' = gi + (b_ih [+ b_hh for r,z]) folded via bias
                        nc.scalar.activation(
                            gi[:, m * CB : (m + 1) * CB],
                            ps_gi[:],
                            AF.Identity,
                            bias=pp(CV_GIB + m),
                        )

            # ---- GRU + stages 2..4; chunks interleaved per step so their
            # independent dependency chains overlap across engines ----
            gin_bf = []
            hs = []
            packs = [None] * NCH
            for c in range(NCH):
                g_bf = gip.tile([128, 2 * CB], gdt, tag=f"ginb{c}", name=f"ginb{c}")
                nc.scalar.activation(g_bf[:], gi_tiles[c][:, 4 * CB : 6 * CB], AF.Copy)
                gin_bf.append(g_bf)
                h = hp.tile([128, 2 * CB], gdt, tag=f"h{c}")
                nc.vector.memset(h[:], 0.0)
                hs.append(h)
            w2s = [None] * NCH
            y2s = [None] * NCH

            def emit_conv(t, c):
                """conv matmuls for step t (emitted at period t+1: inputs ready)"""
                ps_cv = ps2.tile([128, 2 * CB], F32, tag="g", name=f"pscv{c}")
                w2 = w2s[c]
                for k in range(2):
                    for m in range(2):
                        nc.tensor.matmul(
                            ps_cv[:, m * CB : (m + 1) * CB],
                            wc_t[:, k * H + m * 128 : k * H + (m + 1) * 128],
                            w2[:, k * CB : (k + 1) * CB],
                            start=(k == 0),
                            stop=(k == 1),
                        )
                return ps_cv

            def emit_prelu3(ps_cv, c):
                y2 = gp.tile([128, 2 * CB], gdt, tag="y2", name=f"y2_{c}")
                for m in range(2):
                    nc.scalar.activation(
                        y2[:, m * CB : (m + 1) * CB],
                        ps_cv[:, m * CB : (m + 1) * CB],
                        AF.Prelu,
                        bias=pp(CV_T3 + m),
                        scale=pp(CV_S3 + m),
                        alpha=pp(CV_A3),
                    )
                y2s[c] = y2

            def emit_wmu(t, c):
                """W_mu head for step t (emitted at period t+2), col-tiled so 4
                consecutive steps pack one PSUM bank [4l x 32c, b]"""
                j = t % 4
                if j == 0:
                    packs[c] = ps1.tile([128, CB], F32, tag="p", name=f"pack{c}")
                ps_pack = packs[c]
                y2 = y2s[c]
                for k in range(2):
                    nc.tensor.matmul(
                        ps_pack[32 * j : 32 * (j + 1), :],
                        wmu_t[:, k * C : (k + 1) * C],
                        y2[:, k * CB : (k + 1) * CB],
                        start=(k == 0),
                        stop=(k == 1),
                        tile_position=(0, 32 * j),
                    )

            def emit_pack_out(t, c):
                if t % 4 != 3:
                    return
                ps_pack = packs[c]
                pk = sp.tile([128, CB], F32, tag="pk", name=f"pk{c}")
                nc.scalar.activation(pk[:], ps_pack[:], AF.Identity, bias=pp(CV_BMU))
                tr = sp.tile([128, CB], F32, tag="tr", name=f"tr{c}")
                nc.vector.transpose(tr[:], pk[:])
                l4 = t // 4
                dst = out_h[:].rearrange(
                    "(c bh bl) (l4 li cc) -> c l4 li bl bh cc",
                    c=NCH, bh=CB // 32, bl=32, l4=L // 4, li=4, cc=C,
                )
                for li in range(4):
                    nc.sync.dma_start(
                        dst[c, l4, li],
                        tr[32 * li : 32 * (li + 1), :].rearrange(
                            "p (bh cc) -> p bh cc", bh=CB // 32
                        ),
                    )

            def emit_gate_mms(g, psl):
                # preloads then real matmuls for one gate, both chunks; gate
                # order r,z,n lets the ACT/DVE chain start while n still runs
                for c in range(NCH):
                    if g < 2:
                        for m in range(2):
                            nc.tensor.matmul(
                                psl[c][:, m * CB : (m + 1) * CB],
                                idt[:],
                                gi_tiles[c][:, (2 * g + m) * CB : (2 * g + m + 1) * CB],
                                start=True,
                                stop=False,
                            )
                for c in range(NCH):
                    for k in range(2):
                        for m in range(2):
                            row = 2 * g + m
                            nc.tensor.matmul(
                                psl[c][:, m * CB : (m + 1) * CB],
                                whh_t[:, k * 3 * H + row * 128 : k * 3 * H + (row + 1) * 128],
                                hs[c][:, k * CB : (k + 1) * CB],
                                start=(g == 2 and k == 0),
                                stop=(k == 1),
                            )

            for t in range(L):
                psr, psz, psn = [], [], []
                for c in range(NCH):
                    psr.append(ps2.tile([128, 2 * CB], F32, tag="g", name=f"psr{c}"))
                    psz.append(ps2.tile([128, 2 * CB], F32, tag="g", name=f"psz{c}"))
                    psn.append(ps2.tile([128, 2 * CB], F32, tag="g", name=f"psn{c}"))
                emit_gate_mms(0, psr)
                emit_gate_mms(1, psz)
                emit_gate_mms(2, psn)
                # ready PE work from earlier steps fills the PE while this
                # step's gate chain serializes on ACT/DVE: conv of t-1 (its w2
                # is done), W_mu of t-2 (its prelu3 is done)
                pscs = [None] * NCH
                if t >= 1:
                    for c in range(NCH):
                        pscs[c] = emit_conv(t - 1, c)
                if t >= 2:
                    for c in range(NCH):
                        emit_wmu(t - 2, c)
                # gate chains, op-interleaved across chunks so ACT and DVE
                # always have the other chunk's op available
                r_sb, z_sb, t_sb, a_sb, n_sb, u_sb, v_sb = ({} for _ in range(7))
                for c in range(NCH):
                    r_sb[c] = gp.tile([128, 2 * CB], gdt, tag="r", name=f"r{c}")
                    nc.scalar.activation(r_sb[c][:], psr[c][:], AF.Sigmoid)
                for c in range(NCH):
                    z_sb[c] = gp.tile([128, 2 * CB], gdt, tag="z", name=f"z{c}")
                    nc.scalar.activation(z_sb[c][:], psz[c][:], AF.Sigmoid)
                for c in range(NCH):
                    t_sb[c] = gp.tile([128, 2 * CB], gdt, tag="t", name=f"t{c}")
                    if BHN_ZERO:
                        # b_hh_n == 0: single fused multiply over both rows
                        nc.vector.tensor_tensor(
                            t_sb[c][:], psn[c][:], r_sb[c][:], OP.mult
                        )
                    else:
                        for m in range(2):
                            # (gh_n + b_hh_n) * r
                            nc.vector.scalar_tensor_tensor(
                                t_sb[c][:, m * CB : (m + 1) * CB],
                                psn[c][:, m * CB : (m + 1) * CB],
                                pp(CV_BHN + m),
                                r_sb[c][:, m * CB : (m + 1) * CB],
                                op0=OP.add,
                                op1=OP.mult,
                            )
                for c in range(NCH):
                    a_sb[c] = gp.tile([128, 2 * CB], gdt, tag="a", name=f"a{c}")
                    nc.vector.tensor_tensor(a_sb[c][:], t_sb[c][:], gin_bf[c][:], OP.add)
                for c in range(NCH):
                    n_sb[c] = gp.tile([128, 2 * CB], gdt, tag="n", name=f"n{c}")
                    nc.scalar.activation(n_sb[c][:], a_sb[c][:], AF.Tanh)
                for c in range(NCH):
                    u_sb[c] = gp.tile([128, 2 * CB], gdt, tag="u", name=f"u{c}")
                    nc.vector.tensor_tensor(u_sb[c][:], hs[c][:], n_sb[c][:], OP.subtract)
                for c in range(NCH):
                    v_sb[c] = gp.tile([128, 2 * CB], gdt, tag="v", name=f"v{c}")
                    nc.vector.tensor_tensor(v_sb[c][:], z_sb[c][:], u_sb[c][:], OP.mult)
                for c in range(NCH):
                    # h' = n + z*(h - n)
                    h = hp.tile([128, 2 * CB], gdt, tag=f"h{c}", name=f"h{c}")
                    nc.vector.tensor_tensor(h[:], n_sb[c][:], v_sb[c][:], OP.add)
                    hs[c] = h
                for c in range(NCH):
                    # stage 2 (off the recurrence): w2 = prelu(bn2(h_t)) on ACT
                    w2 = gp.tile([128, 2 * CB], gdt, tag="w2", name=f"w2_{c}")
                    for m in range(2):
                        nc.scalar.activation(
                            w2[:, m * CB : (m + 1) * CB],
                            hs[c][:, m * CB : (m + 1) * CB],
                            AF.Prelu,
                            bias=pp(CV_T2 + m),
                            scale=pp(CV_S2 + m),
                            alpha=pp(CV_A2),
                        )
                    w2s[c] = w2
                # ACT/DVE tails of the stage pipeline
                if t >= 1:
                    for c in range(NCH):
                        emit_prelu3(pscs[c], c)
                if t >= 2:
                    for c in range(NCH):
                        emit_pack_out(t - 2, c)
            # epilogue: drain the stage pipeline
            for c in range(NCH):
                psc = emit_conv(L - 1, c)
                emit_wmu(L - 2, c)
                emit_prelu3(psc, c)
                emit_pack_out(L - 2, c)
            for c in range(NCH):
                emit_wmu(L - 1, c)
                emit_pack_out(L - 1, c)

    nc.compile()
    return nc


def _prep_inputs(inputs):
    f32 = np.float32
    gnp = _np_gdt()
    x = np.ascontiguousarray(np.asarray(inputs["h_w_action"], f32).reshape(B, F))
    W_lin = np.asarray(inputs["W_lin"], f32)
    b_lin = np.asarray(inputs["b_lin"], f32)
    W_ih = np.asarray(inputs["W_ih"], f32)
    W_hh = np.asarray(inputs["W_hh"], f32)
    b_ih = np.asarray(inputs["b_ih"], f32)
    b_hh = np.asarray(inputs["b_hh"], f32)
    Wc = np.asarray(inputs["Wc"], f32)
    bc = np.asarray(inputs["bc"], f32)
    W_mu = np.asarray(inputs["W_mu"], f32)
    b_mu = np.asarray(inputs["b_mu"], f32)

    def bnfold(g, beta, m, v):
        s = g / np.sqrt(v + EPS)
        return s, beta - m * s

    s1, t1 = bnfold(inputs["g1"], inputs["beta1"], inputs["m1"], inputs["v1"])
    s2, t2 = bnfold(inputs["g2"], inputs["beta2"], inputs["m2"], inputs["v2"])
    s3, t3 = bnfold(inputs["g3"], inputs["beta3"], inputs["m3"], inputs["v3"])
    s1, t1, s2, t2, s3, t3 = (np.asarray(a, f32) for a in (s1, t1, s2, t2, s3, t3))
    t1 = t1 + s1 * b_lin          # fold linear bias into bn1 shift
    t3 = t3 + s3 * bc             # fold conv bias into bn3 shift
    gib = b_ih.copy()
    gib[: 2 * H] += b_hh[: 2 * H]  # fold b_hh into gi for the r,z gates
    bhn = b_hh[2 * H :]

    cv = np.zeros((128, NV), f32)
    for col, vec in ((CV_S1, s1), (CV_T1, t1), (CV_S2, s2), (CV_T2, t2),
                     (CV_S3, s3), (CV_T3, t3)):
        cv[:, col] = vec[:128]
        cv[:, col + 1] = vec[128:]
    for m in range(6):
        cv[:, CV_GIB + m] = gib[m * 128 : (m + 1) * 128]
    cv[:, CV_BHN] = bhn[:128]
    cv[:, CV_BHN + 1] = bhn[128:]
    cv[:, CV_BMU] = np.tile(b_mu, 4)
    cv[:, CV_A1] = f32(np.asarray(inputs["a1"]).reshape(-1)[0])
    cv[:, CV_A2] = f32(np.asarray(inputs["a2"]).reshape(-1)[0])
    cv[:, CV_A3] = f32(np.asarray(inputs["a3"]).reshape(-1)[0])

    shared = {
        "wlin": np.ascontiguousarray(W_lin.T),
        "wih": np.ascontiguousarray(W_ih.T),
        "whh": np.ascontiguousarray(W_hh.T).astype(gnp),
        "wc": np.ascontiguousarray(Wc.T).astype(gnp),
        "wmu": np.ascontiguousarray(W_mu.T).astype(gnp),
        "cv": cv,
        "idt": np.eye(128, dtype=f32),
    }
    in_maps = []
    for i in range(NCORES):
        m = dict(shared)
        m["xT"] = np.ascontiguousarray(x[i * BC : (i + 1) * BC, :].T)
        in_maps.append(m)
    return in_maps


def kernel(**inputs) -> np.ndarray:
    bhn_zero = bool(np.all(np.asarray(inputs["b_hh"])[2 * H :] == 0))
    key = ("nc", bhn_zero)
    if key not in _CACHE:
        _CACHE[key] = build_program(bhn_zero)
    nc = _CACHE[key]
    _CACHE["last"] = nc
    in_maps = _prep_inputs(inputs)
    res = bass_utils.run_bass_kernel_spmd(nc, in_maps, core_ids=list(range(NCORES)))
    outs = [np.asarray(r["out"], np.float32) for r in res.results]
    return np.concatenate(outs, axis=0).reshape(E, S, L, C)


# revision 17
# speedup vs baseline: 2.0742x; 1.0208x over previous
"""Trainium2 Bass kernel for nn_Comm_OUT (Linear+BN+PReLU -> 20-step GRU ->
BN+PReLU -> 1x1 conv -> BN+PReLU -> linear head), data-parallel over 8 cores.

Layout strategy: everything on-chip is kept "transposed" (channels on SBUF
partitions, batch on the free dim) so the GRU recurrence never needs an
on-chip transpose:
    gh.T[3H, B] = W_hh @ h.T   (W_hh.T tiles are the stationary operand)
The per-step `gi + gh` adds are folded into the PE via identity-matmul PSUM
preloads, biases are folded into per-partition activation bias operands, and
BN+PReLU collapses into single Prelu activations.
"""

import numpy as np
import ml_dtypes

import concourse.bacc as bacc
import concourse.mybir as mybir
import concourse.tile as tile
from concourse import bass_utils

AF = mybir.ActivationFunctionType
OP = mybir.AluOpType
F32 = mybir.dt.float32
F32R = mybir.dt.float32r
BF16 = mybir.dt.bfloat16

E, S, F, H, C, L = 64, 128, 640, 256, 32, 20
EPS = 1e-5
NCORES = 8
B = E * S              # 8192
BC = B // NCORES       # 1024 batch rows per core
NCH = 2                # chunks per core
CB = BC // NCH         # 512 batch rows per chunk (PSUM-bank friendly)

# --- precision config -------------------------------------------------------
# GATE_BF16: store gates/hidden state (and conv/W_mu operands) in bf16 for 2x
# DVE tensor_tensor throughput. Matmuls otherwise run fp32 bitcast to fp32r
# (full PE speed at N>=256).
GATE_BF16 = True

# const-vector column indices (packed [128, NV] tensor, one column per
# per-partition operand vector)
CV_S1, CV_T1, CV_GIB, CV_BHN, CV_S2, CV_T2, CV_S3, CV_T3 = 0, 2, 4, 10, 12, 14, 16, 18
CV_BMU, CV_A1, CV_A2, CV_A3 = 20, 21, 22, 23
NV = 24

_CACHE: dict = {}


def _gdt():
    return BF16 if GATE_BF16 else F32


def _np_gdt():
    return ml_dtypes.bfloat16 if GATE_BF16 else np.float32


def _mm(x):
    """bitcast fp32 APs to fp32r for full-rate PE; bf16 passes through."""
    if x.dtype == F32:
        return x.bitcast(F32R)
    return x


def build_program(bhn_zero=True):
    BHN_ZERO = bhn_zero
    gdt = _gdt()
    nc = bacc.Bacc("TRN2", target_bir_lowering=False, debug=False)

    xT_h = nc.dram_tensor("xT", [F, BC], F32R, kind="ExternalInput")
    wlin_h = nc.dram_tensor("wlin", [F, H], F32R, kind="ExternalInput")
    wih_h = nc.dram_tensor("wih", [H, 3 * H], F32R, kind="ExternalInput")
    whh_h = nc.dram_tensor("whh", [H, 3 * H], gdt, kind="ExternalInput")
    wc_h = nc.dram_tensor("wc", [H, H], gdt, kind="ExternalInput")
    wmu_h = nc.dram_tensor("wmu", [H, C], gdt, kind="ExternalInput")
    cv_h = nc.dram_tensor("cv", [128, NV], F32, kind="ExternalInput")
    idt_h = nc.dram_tensor("idt", [128, 128], F32R, kind="ExternalInput")
    out_h = nc.dram_tensor("out", [BC, L * C], F32, kind="ExternalOutput")

    with tile.TileContext(nc) as tc:
        with (
            tc.tile_pool(name="consts", bufs=1) as cpool,
            tc.tile_pool(name="gi", bufs=1) as gip,
            tc.tile_pool(name="hp", bufs=2) as hp,
            tc.tile_pool(name="gates", bufs=3) as gp,
            tc.tile_pool(name="s24", bufs=2) as sp,
            tc.tile_pool(name="ps2", bufs=3, space="PSUM") as ps2,
            tc.tile_pool(name="ps1", bufs=2, space="PSUM") as ps1,
        ):
            cvt = cpool.tile([128, NV], F32, tag="cv")
            nc.sync.dma_start(cvt[:], cv_h[:])
            idt = cpool.tile([128, 128], F32R, tag="idt")
            nc.sync.dma_start(idt[:], idt_h[:])
            wih_t = cpool.tile([128, 2 * 3 * H], F32R, tag="wih")
            whh_t = cpool.tile([128, 2 * 3 * H], gdt, tag="whh")
            for k in range(2):
                nc.sync.dma_start(
                    wih_t[:, k * 3 * H : (k + 1) * 3 * H],
                    wih_h[k * 128 : (k + 1) * 128, :],
                )
                nc.sync.dma_start(
                    whh_t[:, k * 3 * H : (k + 1) * 3 * H],
                    whh_h[k * 128 : (k + 1) * 128, :],
                )
            wc_t = cpool.tile([128, 2 * H], gdt, tag="wc")
            wmu_t = cpool.tile([128, 2 * C], gdt, tag="wmu")
            for k in range(2):
                nc.sync.dma_start(
                    wc_t[:, k * H : (k + 1) * H], wc_h[k * 128 : (k + 1) * 128, :]
                )
                nc.sync.dma_start(
                    wmu_t[:, k * C : (k + 1) * C], wmu_h[k * 128 : (k + 1) * 128, :]
                )

            def pp(col):  # per-partition operand column
                return cvt[:, col : col + 1]

            # ---- stage 1: x1 = prelu(bn(x @ W_lin.T)), gi = x1 @ W_ih.T ----
            gi_tiles = []
            with tc.tile_pool(name="stage1", bufs=1) as xp:
                xt = xp.tile([128, 5 * BC], F32R, tag="xT")
                for k in range(5):
                    nc.sync.dma_start(
                        xt[:, k * BC : (k + 1) * BC], xT_h[k * 128 : (k + 1) * 128, :]
                    )
                wlin_t = xp.tile([128, 5 * H], F32R, tag="wlin")
                for k in range(5):
                    nc.sync.dma_start(
                        wlin_t[:, k * H : (k + 1) * H], wlin_h[k * 128 : (k + 1) * 128, :]
                    )
                for c in range(NCH):
                    ps_x1 = ps2.tile([128, 2 * CB], F32, tag="g")
                    for k in range(5):
                        for m in range(2):
                            nc.tensor.matmul(
                                ps_x1[:, m * CB : (m + 1) * CB],
                                wlin_t[:, k * H + m * 128 : k * H + (m + 1) * 128],
                                xt[:, k * BC + c * CB : k * BC + (c + 1) * CB],
                                start=(k == 0),
                                stop=(k == 4),
                            )
                    x1p = xp.tile([128, 2 * CB], F32R, tag=f"x1p{c}")
                    for m in range(2):
                        nc.scalar.activation(
                            x1p[:, m * CB : (m + 1) * CB],
                            ps_x1[:, m * CB : (m + 1) * CB],
                            AF.Prelu,
                            bias=pp(CV_T1 + m),
                            scale=pp(CV_S1 + m),
                            alpha=pp(CV_A1),
                        )
                    gi = gip.tile([128, 6 * CB], F32R, tag=f"gi{c}")
                    gi_tiles.append(gi)
                    for m in range(6):
                        ps_gi = ps1.tile([128, CB], F32, tag="p")
                        for k in range(2):
                            nc.tensor.matmul(
                                ps_gi[:],
                                wih_t[:, k * 3 * H + m * 128 : k * 3 * H + (m + 1) * 128],
                                x1p[:, k * CB : (k + 1) * CB],
                                start=(k == 0),
                                stop=(k == 1),
                            )
                        # gi' = gi + (b_ih [+ b_hh for r,z]) folded via bias
                        nc.scalar.activation(
                            gi[:, m * CB : (m + 1) * CB],
                            ps_gi[:],
                            AF.Identity,
                            bias=pp(CV_GIB + m),
                        )

            # ---- GRU + stages 2..4; chunks interleaved per step so their
            # independent dependency chains overlap across engines ----
            gin_bf = []
            hs = []
            packs = [None] * NCH
            for c in range(NCH):
                g_bf = gip.tile([128, 2 * CB], gdt, tag=f"ginb{c}", name=f"ginb{c}")
                nc.scalar.activation(g_bf[:], gi_tiles[c][:, 4 * CB : 6 * CB], AF.Copy)
                gin_bf.append(g_bf)
                h = hp.tile([128, 2 * CB], gdt, tag=f"h{c}")
                nc.vector.memset(h[:], 0.0)
                hs.append(h)
            w2s = [None] * NCH
            y2s = [None] * NCH

            def emit_conv(t, c):
                """conv matmuls for step t (emitted at period t+1: inputs ready)"""
                ps_cv = ps2.tile([128, 2 * CB], F32, tag="g", name=f"pscv{c}")
                w2 = w2s[c]
                for k in range(2):
                    for m in range(2):
                        nc.tensor.matmul(
                            ps_cv[:, m * CB : (m + 1) * CB],
                            wc_t[:, k * H + m * 128 : k * H + (m + 1) * 128],
                            w2[:, k * CB : (k + 1) * CB],
                            start=(k == 0),
                            stop=(k == 1),
                        )
                return ps_cv

            def emit_prelu3(ps_cv, c):
                y2 = gp.tile([128, 2 * CB], gdt, tag="y2", name=f"y2_{c}")
                for m in range(2):
                    nc.scalar.activation(
                        y2[:, m * CB : (m + 1) * CB],
                        ps_cv[:, m * CB : (m + 1) * CB],
                        AF.Prelu,
                        bias=pp(CV_T3 + m),
                        scale=pp(CV_S3 + m),
                        alpha=pp(CV_A3),
                    )
                y2s[c] = y2

            def emit_wmu(t, c):
                """W_mu head for step t (emitted at period t+2), col-tiled so 4
                consecutive steps pack one PSUM bank [4l x 32c, b]"""
                j = t % 4
                if j == 0:
                    packs[c] = ps1.tile([128, CB], F32, tag="p", name=f"pack{c}")
                ps_pack = packs[c]
                y2 = y2s[c]
                for k in range(2):
                    nc.tensor.matmul(
                        ps_pack[32 * j : 32 * (j + 1), :],
                        wmu_t[:, k * C : (k + 1) * C],
                        y2[:, k * CB : (k + 1) * CB],
                        start=(k == 0),
                        stop=(k == 1),
                        tile_position=(0, 32 * j),
                    )

            def emit_pack_out(t, c):
                if t % 4 != 3:
                    return
                ps_pack = packs[c]
                pk = sp.tile([128, CB], F32, tag="pk", name=f"pk{c}")
                nc.scalar.activation(pk[:], ps_pack[:], AF.Identity, bias=pp(CV_BMU))
                tr = sp.tile([128, CB], F32, tag="tr", name=f"tr{c}")
                nc.vector.transpose(tr[:], pk[:])
                l4 = t // 4
                dst = out_h[:].rearrange(
                    "(c bh bl) (l4 li cc) -> c l4 li bl bh cc",
                    c=NCH, bh=CB // 32, bl=32, l4=L // 4, li=4, cc=C,
                )
                for li in range(4):
                    nc.sync.dma_start(
                        dst[c, l4, li],
                        tr[32 * li : 32 * (li + 1), :].rearrange(
                            "p (bh cc) -> p bh cc", bh=CB // 32
                        ),
                    )

            def emit_gate_mms(g, psl):
                # preloads then real matmuls for one gate, both chunks; gate
                # order r,z,n lets the ACT/DVE chain start while n still runs
                for c in range(NCH):
                    if g < 2:
                        for m in range(2):
                            nc.tensor.matmul(
                                psl[c][:, m * CB : (m + 1) * CB],
                                idt[:],
                                gi_tiles[c][:, (2 * g + m) * CB : (2 * g + m + 1) * CB],
                                start=True,
                                stop=False,
                            )
                for c in range(NCH):
                    for k in range(2):
                        for m in range(2):
                            row = 2 * g + m
                            nc.tensor.matmul(
                                psl[c][:, m * CB : (m + 1) * CB],
                                whh_t[:, k * 3 * H + row * 128 : k * 3 * H + (row + 1) * 128],
                                hs[c][:, k * CB : (k + 1) * CB],
                                start=(g == 2 and k == 0),
                                stop=(k == 1),
                            )

            for t in range(L):
                psr, psz, psn = [], [], []
                for c in range(NCH):
                    psr.append(ps2.tile([128, 2 * CB], F32, tag="g", name=f"psr{c}"))
                    psz.append(ps2.tile([128, 2 * CB], F32, tag="g", name=f"psz{c}"))
                    psn.append(ps2.tile([128, 2 * CB], F32, tag="g", name=f"psn{c}"))
                emit_gate_mms(0, psr)
                emit_gate_mms(1, psz)
                emit_gate_mms(2, psn)
                # ready PE work from earlier steps fills the PE while this
                # step's gate chain serializes on ACT/DVE: conv of t-1 (its w2
                # is done), W_mu of t-2 (its prelu3 is done)
                pscs = [None] * NCH
                if t >= 1:
                    for c in range(NCH):
                        pscs[c] = emit_conv(t - 1, c)
                if t >= 2:
                    for c in range(NCH):
                        emit_wmu(t - 2, c)
                # gate chains, op-interleaved across chunks so ACT and DVE
                # always have the other chunk's op available
                r_sb, z_sb, t_sb, a_sb, n_sb, u_sb, v_sb = ({} for _ in range(7))
                for c in range(NCH):
                    r_sb[c] = gp.tile([128, 2 * CB], gdt, tag="r", name=f"r{c}")
                    nc.scalar.activation(r_sb[c][:], psr[c][:], AF.Sigmoid)
                for c in range(NCH):
                    z_sb[c] = gp.tile([128, 2 * CB], gdt, tag="z", name=f"z{c}")
                    nc.scalar.activation(z_sb[c][:], psz[c][:], AF.Sigmoid)
                for c in range(NCH):
                    t_sb[c] = gp.tile([128, 2 * CB], gdt, tag="t", name=f"t{c}")
                    if BHN_ZERO:
                        # b_hh_n == 0: single fused multiply over both rows
                        nc.vector.tensor_tensor(
                            t_sb[c][:], psn[c][:], r_sb[c][:], OP.mult
                        )
                    else:
                        for m in range(2):
                            # (gh_n + b_hh_n) * r
                            nc.vector.scalar_tensor_tensor(
                                t_sb[c][:, m * CB : (m + 1) * CB],
                                psn[c][:, m * CB : (m + 1) * CB],
                                pp(CV_BHN + m),
                                r_sb[c][:, m * CB : (m + 1) * CB],
                                op0=OP.add,
                                op1=OP.mult,
                            )
                for c in range(NCH):
                    a_sb[c] = gp.tile([128, 2 * CB], gdt, tag="a", name=f"a{c}")
                    nc.vector.tensor_tensor(a_sb[c][:], t_sb[c][:], gin_bf[c][:], OP.add)
                for c in range(NCH):
                    n_sb[c] = gp.tile([128, 2 * CB], gdt, tag="n", name=f"n{c}")
                    nc.scalar.activation(n_sb[c][:], a_sb[c][:], AF.Tanh)
                for c in range(NCH):
                    u_sb[c] = gp.tile([128, 2 * CB], gdt, tag="u", name=f"u{c}")
                    nc.vector.tensor_tensor(u_sb[c][:], hs[c][:], n_sb[c][:], OP.subtract)
                for c in range(NCH):
                    v_sb[c] = gp.tile([128, 2 * CB], gdt, tag="v", name=f"v{c}")
                    nc.vector.tensor_tensor(v_sb[c][:], z_sb[c][:], u_sb[c][:], OP.mult)
                for c in range(NCH):
                    # h' = n + z*(h - n)
                    h = hp.tile([128, 2 * CB], gdt, tag=f"h{c}", name=f"h{c}")
                    nc.vector.tensor_tensor(h[:], n_sb[c][:], v_sb[c][:], OP.add)
                    hs[c] = h
                for c in range(NCH):
                    # stage 2 (off the recurrence): w2 = prelu(bn2(h_t)) on DVE
                    w2a = gp.tile([128, 2 * CB], gdt, tag="w2a", name=f"w2a{c}")
                    for m in range(2):
                        nc.vector.tensor_scalar(
                            w2a[:, m * CB : (m + 1) * CB],
                            hs[c][:, m * CB : (m + 1) * CB],
                            pp(CV_S2 + m),
                            pp(CV_T2 + m),
                            op0=OP.mult,
                            op1=OP.add,
                        )
                    w2 = gp.tile([128, 2 * CB], gdt, tag="w2", name=f"w2_{c}")
                    # prelu(x) = max(a*x, x) for 0 <= a <= 1
                    nc.vector.scalar_tensor_tensor(
                        w2[:], w2a[:], pp(CV_A2), w2a[:], op0=OP.mult, op1=OP.max
                    )
                    w2s[c] = w2
                # ACT/DVE tails of the stage pipeline
                if t >= 1:
                    for c in range(NCH):
                        emit_prelu3(pscs[c], c)
                if t >= 2:
                    for c in range(NCH):
                        emit_pack_out(t - 2, c)
            # epilogue: drain the stage pipeline
            for c in range(NCH):
                psc = emit_conv(L - 1, c)
                emit_wmu(L - 2, c)
                emit_prelu3(psc, c)
                emit_pack_out(L - 2, c)
            for c in range(NCH):
                emit_wmu(L - 1, c)
                emit_pack_out(L - 1, c)

    nc.compile()
    return nc


def _prep_inputs(inputs):
    f32 = np.float32
    gnp = _np_gdt()
    x = np.ascontiguousarray(np.asarray(inputs["h_w_action"], f32).reshape(B, F))
    W_lin = np.asarray(inputs["W_lin"], f32)
    b_lin = np.asarray(inputs["b_lin"], f32)
    W_ih = np.asarray(inputs["W_ih"], f32)
    W_hh = np.asarray(inputs["W_hh"], f32)
    b_ih = np.asarray(inputs["b_ih"], f32)
    b_hh = np.asarray(inputs["b_hh"], f32)
    Wc = np.asarray(inputs["Wc"], f32)
    bc = np.asarray(inputs["bc"], f32)
    W_mu = np.asarray(inputs["W_mu"], f32)
    b_mu = np.asarray(inputs["b_mu"], f32)

    def bnfold(g, beta, m, v):
        s = g / np.sqrt(v + EPS)
        return s, beta - m * s

    s1, t1 = bnfold(inputs["g1"], inputs["beta1"], inputs["m1"], inputs["v1"])
    s2, t2 = bnfold(inputs["g2"], inputs["beta2"], inputs["m2"], inputs["v2"])
    s3, t3 = bnfold(inputs["g3"], inputs["beta3"], inputs["m3"], inputs["v3"])
    s1, t1, s2, t2, s3, t3 = (np.asarray(a, f32) for a in (s1, t1, s2, t2, s3, t3))
    t1 = t1 + s1 * b_lin          # fold linear bias into bn1 shift
    t3 = t3 + s3 * bc             # fold conv bias into bn3 shift
    gib = b_ih.copy()
    gib[: 2 * H] += b_hh[: 2 * H]  # fold b_hh into gi for the r,z gates
    bhn = b_hh[2 * H :]

    cv = np.zeros((128, NV), f32)
    for col, vec in ((CV_S1, s1), (CV_T1, t1), (CV_S2, s2), (CV_T2, t2),
                     (CV_S3, s3), (CV_T3, t3)):
        cv[:, col] = vec[:128]
        cv[:, col + 1] = vec[128:]
    for m in range(6):
        cv[:, CV_GIB + m] = gib[m * 128 : (m + 1) * 128]
    cv[:, CV_BHN] = bhn[:128]
    cv[:, CV_BHN + 1] = bhn[128:]
    cv[:, CV_BMU] = np.tile(b_mu, 4)
    cv[:, CV_A1] = f32(np.asarray(inputs["a1"]).reshape(-1)[0])
    cv[:, CV_A2] = f32(np.asarray(inputs["a2"]).reshape(-1)[0])
    cv[:, CV_A3] = f32(np.asarray(inputs["a3"]).reshape(-1)[0])

    shared = {
        "wlin": np.ascontiguousarray(W_lin.T),
        "wih": np.ascontiguousarray(W_ih.T),
        "whh": np.ascontiguousarray(W_hh.T).astype(gnp),
        "wc": np.ascontiguousarray(Wc.T).astype(gnp),
        "wmu": np.ascontiguousarray(W_mu.T).astype(gnp),
        "cv": cv,
        "idt": np.eye(128, dtype=f32),
    }
    in_maps = []
    for i in range(NCORES):
        m = dict(shared)
        m["xT"] = np.ascontiguousarray(x[i * BC : (i + 1) * BC, :].T)
        in_maps.append(m)
    return in_maps


def kernel(**inputs) -> np.ndarray:
    bhn_zero = bool(np.all(np.asarray(inputs["b_hh"])[2 * H :] == 0))
    key = ("nc", bhn_zero)
    if key not in _CACHE:
        _CACHE[key] = build_program(bhn_zero)
    nc = _CACHE[key]
    _CACHE["last"] = nc
    in_maps = _prep_inputs(inputs)
    res = bass_utils.run_bass_kernel_spmd(nc, in_maps, core_ids=list(range(NCORES)))
    outs = [np.asarray(r["out"], np.float32) for r in res.results]
    return np.concatenate(outs, axis=0).reshape(E, S, L, C)
